# revision 1
# baseline (speedup 1.0000x reference)
"""HRNN Trainium2 kernel: 16 encoders (3-layer tanh RNN + FF) -> 4-layer decoder.

Sharding: expert-parallel, 2 encoders per core across 8 cores; decoder
column-sharded (256 of H_DEC / 128 of D_OUT per core) with AllGathers between
decoder layers. Final output assembled on host from per-core [128, T] shards.

Phase 1 (recurrence) keeps all RNN weights resident in SBUF. The only
per-timestep sequential work is the Wh matvec, done weight-stationary
(lhsT = 128x128 Wh tile, rhs = state column). The non-recurrent W_in
contributions are precomputed per layer as batched matmuls over all T.
States are kept as dual-bf16 (hi+lo) so state quantization error vanishes;
weights are bf16 (fast weight load) except tiny W_in0 (fp32, exact).
"""

import sys
import numpy as np

sys.path.insert(0, "/opt/trn_rl_repo")

import ml_dtypes

E = 16
L = 3
D_IN = 32
D = 512
H_FF = 2048
D_ENC = 512
N_DEC = 4
H_DEC = 2048
D_OUT = 1024
T_FULL = 128
N_CORES = 8

E_LOC = E // N_CORES          # 2 encoders per core
DT = D // 128                 # 4 d-tiles
HD_SH = H_DEC // N_CORES      # 256 decoder hidden per core
HD_SHT = HD_SH // 128         # 2 tiles
DO_SH = D_OUT // N_CORES      # 128 output dims per core
NFT = H_FF // 128             # 16 ff tiles
NCAT = (L * D) // 128         # 12 cat tiles
NDK = (E * D_ENC) // 128      # 64 decoder-input k-tiles
NHD = H_DEC // 128            # 16

BF = ml_dtypes.bfloat16


def _tile_kxm(w):
    """[K, M] -> [128, nk*nm*128] with col ((i*nm)+j)*128 : lhsT tile (i,j)."""
    K, M = w.shape
    nk, nm = K // 128, M // 128
    return np.ascontiguousarray(
        w.reshape(nk, 128, nm, 128).transpose(1, 0, 2, 3).reshape(128, nk * nm * 128)
    )


def _bias_cols(b):
    """[M] -> [128, M//128] with col j holding b[j*128:(j+1)*128]."""
    return np.ascontiguousarray(b.reshape(-1, 128).T)


def build_nc(t_steps):
    from concourse import bacc, bass, mybir, tile

    F32 = mybir.dt.float32
    BF16 = mybir.dt.bfloat16
    AF = mybir.ActivationFunctionType
    BYPASS = mybir.AluOpType.bypass
    T = t_steps

    nc = bacc.Bacc(None, num_devices=N_CORES)

    # ---- I/O declarations -------------------------------------------------
    xT = nc.dram_tensor("xT", [D_IN, T], F32, kind="ExternalInput")
    win0 = [nc.dram_tensor(f"win0_{k}", [D_IN, D], F32, kind="ExternalInput")
            for k in range(E_LOC)]
    wh = [nc.dram_tensor(f"wh_{k}", [128, L * DT * DT * 128], BF16, kind="ExternalInput")
          for k in range(E_LOC)]
    win = [nc.dram_tensor(f"win_{k}", [128, (L - 1) * DT * DT * 128], BF16, kind="ExternalInput")
           for k in range(E_LOC)]
    b_rnn = [nc.dram_tensor(f"b_{k}", [128, L * DT], F32, kind="ExternalInput")
             for k in range(E_LOC)]
    wff1 = [nc.dram_tensor(f"wff1_{k}", [128, NCAT * NFT * 128], BF16, kind="ExternalInput")
            for k in range(E_LOC)]
    bff1 = [nc.dram_tensor(f"bff1_{k}", [128, NFT], F32, kind="ExternalInput")
            for k in range(E_LOC)]
    wff2 = [nc.dram_tensor(f"wff2_{k}", [128, NFT * DT * 128], BF16, kind="ExternalInput")
            for k in range(E_LOC)]
    bff2 = [nc.dram_tensor(f"bff2_{k}", [128, DT], F32, kind="ExternalInput")
            for k in range(E_LOC)]
    wd0 = nc.dram_tensor("wd0", [128, NDK * HD_SHT * 128], BF16, kind="ExternalInput")
    bd0 = nc.dram_tensor("bd0", [128, HD_SHT], F32, kind="ExternalInput")
    wdm = [nc.dram_tensor(f"wdm{m}", [128, NHD * HD_SHT * 128], BF16, kind="ExternalInput")
           for m in range(N_DEC - 2)]
    bdm = [nc.dram_tensor(f"bdm{m}", [128, HD_SHT], F32, kind="ExternalInput")
           for m in range(N_DEC - 2)]
    wdo = nc.dram_tensor("wdo", [128, NHD * 128], BF16, kind="ExternalInput")
    bdo = nc.dram_tensor("bdo", [128, 1], F32, kind="ExternalInput")
    y_out = nc.dram_tensor("y_out", [DO_SH, T], F32, kind="ExternalOutput")

    # collective bounce buffers
    ag0_in = nc.dram_tensor("ag0_in", [E_LOC * D_ENC, T], BF16)
    ag0_out = nc.dram_tensor("ag0_out", [E * D_ENC, T], BF16, addr_space="Shared")
    agz_in = [nc.dram_tensor(f"agz_in{m}", [HD_SH, T], BF16) for m in range(N_DEC - 1)]
    agz_out = [nc.dram_tensor(f"agz_out{m}", [H_DEC, T], BF16, addr_space="Shared")
               for m in range(N_DEC - 1)]

    RG = [list(range(N_CORES))]

    def colw(i, j, nm):
        return (i * nm + j) * 128

    with tile.TileContext(nc, num_cores=N_CORES) as tc:
        with (
            tc.tile_pool(name="persist", bufs=1) as persist,
            tc.tile_pool(name="dec_w", bufs=1) as dec_w,
            tc.tile_pool(name="ps_small", bufs=4, space="PSUM") as ps_small,
            tc.tile_pool(name="ps_big", bufs=4, space="PSUM") as ps_big,
            tc.tile_pool(name="tmp", bufs=10) as tmp_pool,
        ):
            # --- persistent small tensors + H buffers
            xT_sb = persist.tile([D_IN, T], F32, name="xT", tag="xT")
            nc.sync.dma_start(xT_sb[:], xT[:])
            win0_sb, b_sb, bff1_sb, bff2_sb, ench_sb = [], [], [], [], []
            hhl = [[None] * L for _ in range(E_LOC)]
            for k in range(E_LOC):
                w0 = persist.tile([D_IN, D], F32, name=f"win0_{k}", tag=f"win0_{k}")
                nc.sync.dma_start(w0[:], win0[k][:])
                win0_sb.append(w0)
                bb = persist.tile([128, L * DT], F32, name=f"b_{k}", tag=f"b_{k}")
                nc.sync.dma_start(bb[:], b_rnn[k][:])
                b_sb.append(bb)
                b1 = persist.tile([128, NFT], F32, name=f"bff1_{k}", tag=f"bff1_{k}")
                nc.sync.dma_start(b1[:], bff1[k][:])
                bff1_sb.append(b1)
                b2 = persist.tile([128, DT], F32, name=f"bff2_{k}", tag=f"bff2_{k}")
                nc.sync.dma_start(b2[:], bff2[k][:])
                bff2_sb.append(b2)
                for l in range(L):
                    hhl[k][l] = persist.tile([128, DT, T, 2], BF16, name=f"hhl_{k}_{l}", tag=f"hhl_{k}_{l}")
                ench_sb.append(persist.tile([128, DT, T], BF16, name=f"enc_{k}", tag=f"enc_{k}"))
            bd0_sb = persist.tile([128, HD_SHT], F32, name="bd0", tag="bd0")
            nc.sync.dma_start(bd0_sb[:], bd0[:])
            bdm_sb = []
            for m in range(N_DEC - 2):
                t_ = persist.tile([128, HD_SHT], F32, name=f"bdm{m}", tag=f"bdm{m}")
                nc.sync.dma_start(t_[:], bdm[m][:])
                bdm_sb.append(t_)
            bdo_sb = persist.tile([128, 1], F32, name="bdo", tag="bdo")
            nc.sync.dma_start(bdo_sb[:], bdo[:])

            # --- decoder mid/out weights: prefetch early (small)
            wdm_sb = []
            for m in range(N_DEC - 2):
                t_ = dec_w.tile([128, NHD * HD_SHT * 128], BF16, name=f"wdm{m}", tag=f"wdm{m}")
                nc.sync.dma_start(t_[:], wdm[m][:])
                wdm_sb.append(t_)
            wdo_sb = dec_w.tile([128, NHD * 128], BF16, name="wdo", tag="wdo")
            nc.sync.dma_start(wdo_sb[:], wdo[:])

            # --- FF1 weights (big): prefetch during phase 1; freed before decoder
            with (
                tc.tile_pool(name="ff1e1", bufs=1) as ff1e1,
                tc.tile_pool(name="ff1e0", bufs=1) as ff1e0,
            ):
                wff1_sb = [None, None]
                wff1_sb[1] = ff1e1.tile([128, NCAT * NFT * 128], BF16, name="wff1_1", tag="wff1_1")
                nc.sync.dma_start(wff1_sb[1][:], wff1[1][:])
                wff1_sb[0] = ff1e0.tile([128, NCAT * NFT * 128], BF16, name="wff1_0", tag="wff1_0")
                nc.sync.dma_start(wff1_sb[0][:], wff1[0][:])

                with tc.tile_pool(name="rnn", bufs=1) as rnn:
                    wh_sb, win_sb = [], []
                    for k in range(E_LOC):
                        t_ = rnn.tile([128, L * DT * DT * 128], BF16, name=f"wh_{k}", tag=f"wh_{k}")
                        nc.sync.dma_start(t_[:], wh[k][:])
                        wh_sb.append(t_)
                        t_ = rnn.tile([128, (L - 1) * DT * DT * 128], BF16, tag=f"win_{k}")
                        nc.sync.dma_start(t_[:], win[k][:])
                        win_sb.append(t_)

                    u_sb = [[None] * L for _ in range(E_LOC)]

                    # u0 = x @ W_in0 + b0  (fp32, exact)
                    for k in range(E_LOC):
                        u_sb[k][0] = rnn.tile([128, DT, T], F32, name=f"u_{k}_0", tag=f"u_{k}_0")
                        for j in range(DT):
                            pu = ps_big.tile([128, T], F32, name="psb", tag="psb")
                            nc.tensor.matmul(pu[:], win0_sb[k][:, j * 128:(j + 1) * 128],
                                             xT_sb[:], start=True, stop=True)
                            nc.scalar.activation(u_sb[k][0][:, j, :], pu[:], AF.Identity,
                                                 bias=b_sb[k][:, j:j + 1])

                    # ---- phase 1: three layer windows ----
                    for l in range(L):
                        if l > 0:
                            # u_l = H_{l-1} @ W_in_l + b_l  (dual-bf16 rhs)
                            for k in range(E_LOC):
                                u_sb[k][l] = rnn.tile([128, DT, T], F32, name=f"u_{k}_{l}", tag=f"u_{k}_{l}")
                                for j in range(DT):
                                    pu = ps_big.tile([128, T], F32, name="psb", tag="psb")
                                    cnt = 0
                                    for i in range(DT):
                                        for hl in range(2):
                                            nc.tensor.matmul(
                                                pu[:],
                                                win_sb[k][:, colw((l - 1) * DT + i, j, DT):
                                                          colw((l - 1) * DT + i, j, DT) + 128],
                                                hhl[k][l - 1][:, i, :, hl],
                                                start=(cnt == 0), stop=(cnt == 2 * DT - 1))
                                            cnt += 1
                                    nc.scalar.activation(
                                        u_sb[k][l][:, j, :], pu[:], AF.Identity,
                                        bias=b_sb[k][:, l * DT + j:l * DT + j + 1])

                        for t in range(T):
                            for k in range(E_LOC):
                                if t == 0:
                                    th = tmp_pool.tile([128, DT], F32, name="tmp", tag="tmp")
                                    nc.scalar.activation(th[:], u_sb[k][l][:, :, 0], AF.Tanh)
                                else:
                                    ps = ps_small.tile([128, DT, 2], F32, name="pss", tag="pss")
                                    for j in range(DT):
                                        for i in range(DT):
                                            nc.tensor.matmul(
                                                ps[:, j, :],
                                                wh_sb[k][:, colw(l * DT + i, j, DT):
                                                         colw(l * DT + i, j, DT) + 128],
                                                hhl[k][l][:, i, t - 1, :],
                                                start=(i == 0), stop=(i == DT - 1))
                                    ta = tmp_pool.tile([128, DT], F32, name="tmp", tag="tmp")
                                    nc.vector.tensor_reduce(
                                        ta[:], ps[:, :, :], mybir.AxisListType.X,
                                        mybir.AluOpType.add)
                                    ts_ = tmp_pool.tile([128, DT], F32, name="tmp", tag="tmp")
                                    nc.vector.tensor_add(ts_[:], ta[:], u_sb[k][l][:, :, t])
                                    th = tmp_pool.tile([128, DT], F32, name="tmp", tag="tmp")
                                    nc.scalar.activation(th[:], ts_[:], AF.Tanh)
                                nc.vector.tensor_copy(hhl[k][l][:, :, t, 0], th[:])
                                nc.vector.tensor_sub(hhl[k][l][:, :, t, 1], th[:],
                                                     hhl[k][l][:, :, t, 0])

                # ---- FF phase (rnn pool closed; ff2/ffs pool opens above ff pools) ----
                with tc.tile_pool(name="post1", bufs=1) as post1:
                    wff2_sb, ffs_sb = [], []
                    for k in range(E_LOC):
                        t_ = post1.tile([128, NFT * DT * 128], BF16, name=f"wff2_{k}", tag=f"wff2_{k}")
                        nc.sync.dma_start(t_[:], wff2[k][:])
                        wff2_sb.append(t_)
                        ffs_sb.append(post1.tile([128, NFT, T], BF16, name=f"ffs_{k}", tag=f"ffs_{k}"))

                    for k in range(E_LOC):
                        for m in range(NFT):
                            pf = ps_big.tile([128, T], F32, name="psb", tag="psb")
                            idx = 0
                            for l in range(L):
                                for j in range(DT):
                                    nc.tensor.matmul(
                                        pf[:],
                                        wff1_sb[k][:, colw(l * DT + j, m, NFT):
                                                   colw(l * DT + j, m, NFT) + 128],
                                        hhl[k][l][:, j, :, 0],
                                        start=(idx == 0), stop=(idx == NCAT - 1))
                                    idx += 1
                            nc.scalar.activation(ffs_sb[k][:, m, :], pf[:],
                                                 AF.Gelu_apprx_tanh,
                                                 bias=bff1_sb[k][:, m:m + 1])
                        for j in range(DT):
                            pf2 = ps_big.tile([128, T], F32, name="psb", tag="psb")
                            for i in range(NFT):
                                nc.tensor.matmul(
                                    pf2[:],
                                    wff2_sb[k][:, colw(i, j, DT):colw(i, j, DT) + 128],
                                    ffs_sb[k][:, i, :],
                                    start=(i == 0), stop=(i == NFT - 1))
                            nc.scalar.activation(ench_sb[k][:, j, :], pf2[:], AF.Identity,
                                                 bias=bff2_sb[k][:, j:j + 1])
                        nc.sync.dma_start(
                            ag0_in[k * D_ENC:(k + 1) * D_ENC, :].rearrange(
                                "(j p) t -> p j t", p=128),
                            ench_sb[k][:, :, :])

            # ---- decoder (ff pools closed; their space is reused) ----
            nc.gpsimd.collective_compute(
                "AllGather", BYPASS, replica_groups=RG,
                ins=[ag0_in[:]], outs=[ag0_out[:]])

            with tc.tile_pool(name="dec_run", bufs=1) as dec_run:
                wd0_sb = dec_run.tile([128, NDK * HD_SHT * 128], BF16, name="wd0", tag="wd0")
                # chunked so decoder matmuls can chase the DMA
                csz = NDK * HD_SHT * 128 // 8
                for ch in range(8):
                    nc.sync.dma_start(wd0_sb[:, ch * csz:(ch + 1) * csz],
                                      wd0[:, ch * csz:(ch + 1) * csz])
                cat_sb = dec_run.tile([128, NDK, T], BF16, name="cat", tag="cat")
                nc.sync.dma_start(cat_sb[:],
                                  ag0_out[:].rearrange("(i p) t -> p i t", p=128))

                zloc = dec_run.tile([128, HD_SHT, T], BF16, name="zloc0", tag="zloc0")
                for j2 in range(HD_SHT):
                    pd = ps_big.tile([128, T], F32, name="psb", tag="psb")
                    for i in range(NDK):
                        nc.tensor.matmul(
                            pd[:],
                            wd0_sb[:, colw(i, j2, HD_SHT):colw(i, j2, HD_SHT) + 128],
                            cat_sb[:, i, :],
                            start=(i == 0), stop=(i == NDK - 1))
                    nc.scalar.activation(zloc[:, j2, :], pd[:], AF.Tanh,
                                         bias=bd0_sb[:, j2:j2 + 1])
                nc.sync.dma_start(
                    agz_in[0][:].rearrange("(j p) t -> p j t", p=128), zloc[:])
                nc.gpsimd.collective_compute(
                    "AllGather", BYPASS, replica_groups=RG,
                    ins=[agz_in[0][:]], outs=[agz_out[0][:]])

                for m in range(N_DEC - 2):
                    zf = dec_run.tile([128, NHD, T], BF16, name="zf", tag="zf")
                    nc.sync.dma_start(
                        zf[:], agz_out[m][:].rearrange("(i p) t -> p i t", p=128))
                    zloc2 = dec_run.tile([128, HD_SHT, T], BF16, name=f"zloc{m + 1}", tag=f"zloc{m + 1}")
                    for j2 in range(HD_SHT):
                        pd = ps_big.tile([128, T], F32, name="psb", tag="psb")
                        for i in range(NHD):
                            nc.tensor.matmul(
                                pd[:],
                                wdm_sb[m][:, colw(i, j2, HD_SHT):colw(i, j2, HD_SHT) + 128],
                                zf[:, i, :],
                                start=(i == 0), stop=(i == NHD - 1))
                        nc.scalar.activation(zloc2[:, j2, :], pd[:], AF.Tanh,
                                             bias=bdm_sb[m][:, j2:j2 + 1])
                    nc.sync.dma_start(
                        agz_in[m + 1][:].rearrange("(j p) t -> p j t", p=128), zloc2[:])
                    nc.gpsimd.collective_compute(
                        "AllGather", BYPASS, replica_groups=RG,
                        ins=[agz_in[m + 1][:]], outs=[agz_out[m + 1][:]])

                zf3 = dec_run.tile([128, NHD, T], BF16, name="zf", tag="zf")
                nc.sync.dma_start(
                    zf3[:], agz_out[N_DEC - 2][:].rearrange("(i p) t -> p i t", p=128))
                py = ps_big.tile([128, T], F32, name="psb", tag="psb")
                for i in range(NHD):
                    nc.tensor.matmul(py[:], wdo_sb[:, i * 128:(i + 1) * 128],
                                     zf3[:, i, :], start=(i == 0), stop=(i == NHD - 1))
                y_sb = dec_run.tile([DO_SH, T], F32, name="ysb", tag="ysb")
                nc.scalar.activation(y_sb[:], py[:], AF.Identity, bias=bdo_sb[:])
                nc.sync.dma_start(y_out[:], y_sb[:])

    nc.compile()
    return nc


def prep_inputs(inputs, t_steps):
    """Build the 8 per-core input maps from full numpy inputs."""
    T = t_steps
    f32 = lambda a: np.asarray(a, np.float32)
    x = f32(inputs["x"])
    W_in0, Wh0, b0 = f32(inputs["W_in0"]), f32(inputs["Wh0"]), f32(inputs["b0"])
    W_in_rest, Wh_rest, b_rest = (f32(inputs["W_in_rest"]), f32(inputs["Wh_rest"]),
                                  f32(inputs["b_rest"]))
    W_ff1, b_ff1 = f32(inputs["W_ff1"]), f32(inputs["b_ff1"])
    W_ff2, b_ff2 = f32(inputs["W_ff2"]), f32(inputs["b_ff2"])
    W_d0, b_d0 = f32(inputs["W_d0"]), f32(inputs["b_d0"])
    W_dmid, b_dmid = f32(inputs["W_dmid"]), f32(inputs["b_dmid"])
    W_dout, b_dout = f32(inputs["W_dout"]), f32(inputs["b_dout"])

    xT = np.ascontiguousarray(x[0, :T].T)  # [32, T]
    in_maps = []
    for c in range(N_CORES):
        m = {"xT": xT}
        for k in range(E_LOC):
            e = E_LOC * c + k
            m[f"win0_{k}"] = np.ascontiguousarray(W_in0[e])
            wh_all = np.concatenate([Wh0[e][None], Wh_rest[e]], 0)  # [3, D, D]
            m[f"wh_{k}"] = _tile_kxm(wh_all.reshape(L * D, D)).astype(BF)
            m[f"win_{k}"] = _tile_kxm(W_in_rest[e].reshape((L - 1) * D, D)).astype(BF)
            b_all = np.concatenate([b0[e][None], b_rest[e]], 0).reshape(-1)
            m[f"b_{k}"] = _bias_cols(b_all)
            m[f"wff1_{k}"] = _tile_kxm(W_ff1[e]).astype(BF)
            m[f"bff1_{k}"] = _bias_cols(b_ff1[e])
            m[f"wff2_{k}"] = _tile_kxm(W_ff2[e]).astype(BF)
            m[f"bff2_{k}"] = _bias_cols(b_ff2[e])
        m["wd0"] = _tile_kxm(W_d0[:, c * HD_SH:(c + 1) * HD_SH]).astype(BF)
        m["bd0"] = _bias_cols(b_d0[c * HD_SH:(c + 1) * HD_SH])
        for mm in range(N_DEC - 2):
            m[f"wdm{mm}"] = _tile_kxm(W_dmid[mm][:, c * HD_SH:(c + 1) * HD_SH]).astype(BF)
            m[f"bdm{mm}"] = _bias_cols(b_dmid[mm][c * HD_SH:(c + 1) * HD_SH])
        m["wdo"] = _tile_kxm(W_dout[:, c * DO_SH:(c + 1) * DO_SH]).astype(BF)
        m["bdo"] = _bias_cols(b_dout[c * DO_SH:(c + 1) * DO_SH])
        in_maps.append(m)
    return in_maps


def run(inputs, t_steps=T_FULL, trace=False):
    from concourse.bass_utils import run_bass_kernel_spmd

    nc = build_nc(t_steps)
    in_maps = prep_inputs(inputs, t_steps)
    res = run_bass_kernel_spmd(nc, in_maps, list(range(N_CORES)), trace=trace)
    parts = [res.results[c]["y_out"] for c in range(N_CORES)]  # each [128, T]
    y = np.concatenate([np.asarray(p, np.float32).T for p in parts], axis=1)
    return y[None], res


def kernel(**inputs):
    y, _ = run(inputs, T_FULL, trace=False)
    return y



# revision 18
# speedup vs baseline: 1.9412x; 1.9412x over previous
"""HRNN Trainium2 kernel v2: 16 encoders (3-layer tanh RNN + FF) -> 4-layer decoder.

Sharding: expert-parallel, 2 encoders/core over 8 cores; decoder column-sharded
(256 of H_DEC, 128 of D_OUT per core) with AllGathers between layers.

Raw-bass implementation (no tile framework): fp16 weights/states throughout,
u-contributions pre-accumulated into PSUM banks so each recurrence step is
16 matmuls (N=1) + 1 strided tanh ACT with a single semaphore pair per
(encoder, step). Batched matmuls (u-precompute, FF1/FF2, decoder) accumulate
N=128 groups in rotating PSUM tiles. Weight DMA, collectives, and mid-kernel
stores overlap compute via dedicated engine streams. Decoder weights reuse the
recurrence-weight SBUF region (semaphore-gated); decoder activations reuse the
FF1-weight region (collective-chain gated).
"""

import sys
import numpy as np

sys.path.insert(0, "/opt/trn_rl_repo")

E = 16
L = 3
D_IN = 32
D = 512
H_FF = 2048
D_ENC = 512
N_DEC = 4
H_DEC = 2048
D_OUT = 1024
T = 128
N_CORES = 8

E_LOC = E // N_CORES          # 2 encoders per core
DT = D // 128                 # 4 d-tiles
HD_SH = H_DEC // N_CORES      # 256 decoder hidden per core
DO_SH = D_OUT // N_CORES      # 128 output dims per core
NFT = H_FF // 128             # 16 ff tiles
NCAT = (L * D) // 128         # 12 cat tiles
NDK = (E * D_ENC) // 128      # 64 decoder-input k-tiles
NHD = H_DEC // 128            # 16
NBC = 2 * NFT + 2 * DT + 2 + 4 + 1  # packed bias cols


def _tile_kxm(w):
    """[K, M] -> [128, nk*nm*128] with col ((i*nm)+j)*128 : lhsT tile (i,j)."""
    K, M = w.shape
    nk, nm = K // 128, M // 128
    return np.ascontiguousarray(
        w.reshape(nk, 128, nm, 128).transpose(1, 0, 2, 3).reshape(128, nk * nm * 128)
    )


def _bias_cols(b):
    """[M] -> [128, M//128] with col j holding b[j*128:(j+1)*128]."""
    return np.ascontiguousarray(b.reshape(-1, 128).T)


class Ctr:
    """Python-side absolute counter for a hardware semaphore."""

    def __init__(self, handle):
        self.h = handle
        self.v = 0

    def inc(self, inst, amt=1):
        inst.then_inc(self.h, amt)
        self.v += amt
        return self.v


# ---- semaphore threshold formulas (must match emission order) -------------
def act_rec_thr(k, l, t):
    # scalar inc order: for l: for t: for k
    return l * T * E_LOC + t * E_LOC + k + 1


def act_g_thr(gg):
    # batched-group ACTs follow all recurrence ACTs, in group order
    return E_LOC * L * T + gg + 1


ST_ENC = [16, 32]
ST_CAT = 48


def st_z(m):
    return 64 + 32 * m


def st_zf(m):
    return 80 + 32 * m


ST_Y = 64 + 32 * (N_DEC - 1)


def build_nc(debug=False):
    from contextlib import ExitStack

    from concourse import bacc, mybir

    F32 = mybir.dt.float32
    FP16 = mybir.dt.float16
    U8 = mybir.dt.uint8
    AF = mybir.ActivationFunctionType
    BYPASS = mybir.AluOpType.bypass

    nc = bacc.Bacc(None, num_devices=N_CORES)

    # ---- DRAM I/O ---------------------------------------------------------
    xT_aug = nc.dram_tensor("xT_aug", [D_IN + 1, T], FP16, kind="ExternalInput")
    ones_row = nc.dram_tensor("ones_row", [1, T], FP16, kind="ExternalInput")
    win0 = [nc.dram_tensor(f"win0_{k}", [D_IN + 1, D], FP16, kind="ExternalInput")
            for k in range(E_LOC)]
    wh = [nc.dram_tensor(f"wh_{k}", [128, L * 16 * 128], FP16, kind="ExternalInput")
          for k in range(E_LOC)]
    win = [nc.dram_tensor(f"win_{k}", [128, (L - 1) * 16 * 128], FP16, kind="ExternalInput")
           for k in range(E_LOC)]
    brow = [nc.dram_tensor(f"brow_{k}", [1, (L - 1) * D], FP16, kind="ExternalInput")
            for k in range(E_LOC)]
    wff1 = [nc.dram_tensor(f"wff1_{k}", [128, NCAT * NFT * 128], FP16, kind="ExternalInput")
            for k in range(E_LOC)]
    wff2 = [nc.dram_tensor(f"wff2_{k}", [128, NFT * DT * 128], FP16, kind="ExternalInput")
            for k in range(E_LOC)]
    bcols = nc.dram_tensor("bcols", [128, NBC], F32, kind="ExternalInput")
    wd0 = nc.dram_tensor("wd0", [128, NDK * 2 * 128], FP16, kind="ExternalInput")
    wdm = [nc.dram_tensor(f"wdm{m}", [128, NHD * 2 * 128], FP16, kind="ExternalInput")
           for m in range(N_DEC - 2)]
    wdo = nc.dram_tensor("wdo", [128, NHD * 128], FP16, kind="ExternalInput")
    y_out = nc.dram_tensor("y_out", [DO_SH, T], F32, kind="ExternalOutput")
    if debug:
        hd = [nc.dram_tensor(f"hd{k}", [128, L * DT * T], FP16, kind="ExternalOutput")
              for k in range(E_LOC)]
        ed = [nc.dram_tensor(f"ed{k}", [128, DT * T], FP16, kind="ExternalOutput")
              for k in range(E_LOC)]
        ffd = [nc.dram_tensor(f"ffd{k}", [128, NFT * T], FP16, kind="ExternalOutput")
               for k in range(E_LOC)]
        catd = nc.dram_tensor("catd", [128, NDK * T], FP16, kind="ExternalOutput")
        zd = nc.dram_tensor("zd", [128, 2 * T], FP16, kind="ExternalOutput")
        whd = nc.dram_tensor("whd", [128, L * 16 * 128], FP16, kind="ExternalOutput")
        w0d = nc.dram_tensor("w0d", [D_IN + 1, D], FP16, kind="ExternalOutput")
        xd = nc.dram_tensor("xd", [D_IN + 1, T], FP16, kind="ExternalOutput")

    ag0_in = nc.dram_tensor("ag0_in", [E_LOC * D_ENC, T], FP16)
    ag0_out = nc.dram_tensor("ag0_out", [E * D_ENC, T], FP16, addr_space="Shared")
    agz_in = [nc.dram_tensor(f"agz_in{m}", [HD_SH, T], FP16) for m in range(N_DEC - 1)]
    agz_out = [nc.dram_tensor(f"agz_out{m}", [H_DEC, T], FP16, addr_space="Shared")
               for m in range(N_DEC - 1)]
    RG = [list(range(N_CORES))]

    def whcol(l, i, j):
        return ((l * 16) + i * 4 + j) * 128

    def wincol(l, i, j):
        return ((l - 1) * 16 * 128) + (i * 4 + j) * 128

    with ExitStack() as ctx:
        ec = ctx.enter_context

        s_ld = Ctr(ec(nc.semaphore("s_ld")))
        s_mm = Ctr(ec(nc.semaphore("s_mm")))
        s_act = Ctr(ec(nc.semaphore("s_act")))
        s_st = Ctr(ec(nc.semaphore("s_st")))
        s_cc = Ctr(ec(nc.semaphore("s_cc")))
        s_dbg = Ctr(ec(nc.semaphore("s_dbg")))

        # ---- SBUF: left = long-lived ------------------------------------
        xT_sb = ec(nc.sbuf_tensor("xT_sb", [D_IN + 1, T], FP16))
        ones_sb = ec(nc.sbuf_tensor("ones_sb", [1, T], FP16))
        win0_sb = [ec(nc.sbuf_tensor(f"win0_sb{k}", [D_IN + 1, D], FP16))
                   for k in range(E_LOC)]
        brow_sb = [ec(nc.sbuf_tensor(f"brow_sb{k}", [1, (L - 1) * D], FP16))
                   for k in range(E_LOC)]
        bc_sb = ec(nc.sbuf_tensor("bc_sb", [128, NBC], F32))
        hb = [ec(nc.sbuf_tensor(f"hb{k}", [128, L * DT * T], FP16))
              for k in range(E_LOC)]
        ffs = [ec(nc.sbuf_tensor(f"ffs{k}", [128, NFT * T], FP16))
               for k in range(E_LOC)]
        enc_sb = [ec(nc.sbuf_tensor(f"enc{k}", [128, DT * T], FP16))
                  for k in range(E_LOC)]
        wff1_sb = [ec(nc.sbuf_tensor(f"wff1_sb{k}", [128, NCAT * NFT * 128], FP16))
                   for k in range(E_LOC)]
        wff2_sb = [ec(nc.sbuf_tensor(f"wff2_sb{k}", [128, NFT * DT * 128], FP16))
                   for k in range(E_LOC)]

        # ---- SBUF: right = 52KB arena, phase-aliased --------------------
        ARENA = 52 * 1024
        arena = ec(nc.sbuf_tensor("arena", [128, ARENA], U8, side="right"))
        abase = nc.lookup_mloc(arena).addr

        def at(name, shape, dtype, off):
            return nc.alloc_sbuf_tensor_at(name, shape, dtype, offset=abase + off)

        # phase 1 (recurrence): wh (12KB x2) + win (8KB x2) = 40KB
        wh_sb = [at(f"wh_sb{k}", [128, L * 16 * 128], FP16, k * 12 * 1024)
                 for k in range(E_LOC)]
        win_sb = [at(f"win_sb{k}", [128, (L - 1) * 16 * 128], FP16,
                     24 * 1024 + k * 8 * 1024) for k in range(E_LOC)]
        # phase 2 (decoder weights; DMA gated on recurrence completion)
        wd0_sb = at("wd0_sb", [128, NDK * 2 * 128], FP16, 0)            # 32KB
        wdm_sb = [at(f"wdm_sb{m}", [128, NHD * 2 * 128], FP16,
                     32 * 1024 + m * 8 * 1024) for m in range(N_DEC - 2)]  # 8KB x2
        wdo_sb = at("wdo_sb", [128, NHD * 128], FP16, 48 * 1024)        # 4KB

        # decoder activations alias the FF1[0] weight region (dead after FF1)
        wbase = nc.lookup_mloc(wff1_sb[0]).addr

        def at_w(name, shape, dtype, off):
            return nc.alloc_sbuf_tensor_at(name, shape, dtype, offset=wbase + off)

        cat_sb = at_w("cat_sb", [128, NDK * T], FP16, 0)                # 16KB
        zf_sb = at_w("zf_sb", [128, NHD * T], FP16, 16 * 1024)          # 4KB
        zloc_sb = at_w("zloc_sb", [128, 2 * T], FP16, 20 * 1024)        # 512B
        ysb = at_w("ysb", [DO_SH, T], F32, 21 * 1024)                   # 512B

        # ---- PSUM -------------------------------------------------------
        pu = [ec(nc.psum_tensor(f"pu{k}", [128, DT * T], F32)) for k in range(E_LOC)]
        psb = [ec(nc.psum_tensor(f"psb{g}", [128, 512], F32)) for g in range(4)]

        def bff1_ap(k, j):
            return bc_sb[:, k * NFT + j:k * NFT + j + 1]

        def bff2_ap(k, j):
            c = 2 * NFT + k * DT + j
            return bc_sb[:, c:c + 1]

        def bd0_ap(j):
            c = 2 * NFT + 2 * DT + j
            return bc_sb[:, c:c + 1]

        def bdm_ap(m, j):
            c = 2 * NFT + 2 * DT + 2 + m * 2 + j
            return bc_sb[:, c:c + 1]

        def bdo_ap():
            c = 2 * NFT + 2 * DT + 2 + 4
            return bc_sb[:, c:c + 1]

        thr = {}

        with nc.Block() as block:

            @block.sync
            def _(sync):
                def load(dst, src):
                    return s_ld.inc(sync.dma_start(dst, src), 16)

                load(xT_sb[:, :], xT_aug[:, :])
                load(ones_sb[:, :], ones_row[:, :])
                load(bc_sb[:, :], bcols[:, :])
                for k in range(E_LOC):
                    load(win0_sb[k][:, :], win0[k][:, :])
                    load(brow_sb[k][:, :], brow[k][:, :])
                for k in range(E_LOC):
                    thr[f"ld_wh{k}"] = load(wh_sb[k][:, :], wh[k][:, :])
                for k in range(E_LOC):
                    thr[f"ld_win{k}"] = load(win_sb[k][:, :], win[k][:, :])
                for k in range(E_LOC):
                    thr[f"ld_wff1{k}"] = load(wff1_sb[k][:, :], wff1[k][:, :])
                for k in range(E_LOC):
                    thr[f"ld_wff2{k}"] = load(wff2_sb[k][:, :], wff2[k][:, :])
                # decoder weights alias wh/win: wait until the recurrence is done
                sync.wait_ge(s_act.h, E_LOC * L * T)
                thr["ld_wd0"] = load(wd0_sb[:, :], wd0[:, :])
                for m in range(N_DEC - 2):
                    load(wdm_sb[m][:, :], wdm[m][:, :])
                thr["ld_wdec"] = load(wdo_sb[:, :], wdo[:, :])

            @block.tensor
            def _(tensor):
                tensor.wait_ge(s_ld.h, thr["ld_wh1"])
                for l in range(L):
                    for k in range(E_LOC):
                        if l == 0:
                            for j in range(DT):
                                mm = tensor.matmul(
                                    pu[k][:, j * T:(j + 1) * T],
                                    win0_sb[k][:, j * 128:(j + 1) * 128],
                                    xT_sb[:, :],
                                    start=(j == 0), stop=True,
                                    skip_group_check=True)
                        else:
                            if k == 0 and l == 1:
                                tensor.wait_ge(s_ld.h, thr["ld_win1"])
                            tensor.wait_ge(s_act.h, act_rec_thr(k, l - 1, T - 1))
                            for j in range(DT):
                                for i in range(DT):
                                    tensor.matmul(
                                        pu[k][:, j * T:(j + 1) * T],
                                        win_sb[k][:, wincol(l, i, j):wincol(l, i, j) + 128],
                                        hb[k][:, ((l - 1) * DT + i) * T:((l - 1) * DT + i + 1) * T],
                                        start=(j == 0 and i == 0), stop=False,
                                        skip_group_check=True)
                                mm = tensor.matmul(
                                    pu[k][:, j * T:(j + 1) * T],
                                    brow_sb[k][0:1, (l - 1) * D + j * 128:(l - 1) * D + (j + 1) * 128],
                                    ones_sb[0:1, :],
                                    start=False, stop=True,
                                    skip_group_check=True)
                        thr[f"mm_u_{k}_{l}"] = s_mm.inc(mm)

                    with nc.named_scope(f"rec{l}"):
                        for t in range(1, T):
                            for k in range(E_LOC):
                                tensor.wait_ge(s_act.h, act_rec_thr(k, l, t - 1))
                                for j in range(DT):
                                    for i in range(DT):
                                        mm = tensor.matmul(
                                            pu[k][:, j * T + t:j * T + t + 1],
                                            wh_sb[k][:, whcol(l, i, j):whcol(l, i, j) + 128],
                                            hb[k][:, (l * DT + i) * T + t - 1:(l * DT + i) * T + t],
                                            start=False, stop=(i == DT - 1),
                                            skip_group_check=True)
                                thr[f"mm_rec_{k}_{l}_{t}"] = s_mm.inc(mm)

                # --- FF1 -------------------------------------------------
                g = 0
                tensor.wait_ge(s_ld.h, thr["ld_wff11"])
                with nc.named_scope("ff1"):
                    for k in range(E_LOC):
                        tensor.wait_ge(s_act.h, act_rec_thr(k, L - 1, T - 1))
                        for j in range(NFT):
                            if g >= 4:
                                tensor.wait_ge(s_act.h, act_g_thr(g - 4))
                            for i in range(NCAT):
                                mm = tensor.matmul(
                                    psb[g % 4][:, 0:T],
                                    wff1_sb[k][:, (i * NFT + j) * 128:(i * NFT + j) * 128 + 128],
                                    hb[k][:, i * T:(i + 1) * T],
                                    start=(i == 0), stop=(i == NCAT - 1))
                            thr[f"mm_g{g}"] = s_mm.inc(mm)
                            g += 1

                # --- FF2 -------------------------------------------------
                tensor.wait_ge(s_ld.h, thr["ld_wff21"])
                with nc.named_scope("ff2"):
                    for k in range(E_LOC):
                        tensor.wait_ge(s_act.h, act_g_thr(k * NFT + NFT - 1))
                        for j in range(DT):
                            if g >= 4:
                                tensor.wait_ge(s_act.h, act_g_thr(g - 4))
                            for i in range(NFT):
                                mm = tensor.matmul(
                                    psb[g % 4][:, 0:T],
                                    wff2_sb[k][:, (i * DT + j) * 128:(i * DT + j) * 128 + 128],
                                    ffs[k][:, i * T:(i + 1) * T],
                                    start=(i == 0), stop=(i == NFT - 1))
                            thr[f"mm_g{g}"] = s_mm.inc(mm)
                            g += 1

                # --- decoder ---------------------------------------------
                tensor.wait_ge(s_ld.h, thr["ld_wd0"])
                tensor.wait_ge(s_st.h, ST_CAT)
                with nc.named_scope("dec"):
                    for j in range(2):
                        if g >= 4:
                            tensor.wait_ge(s_act.h, act_g_thr(g - 4))
                        for i in range(NDK):
                            mm = tensor.matmul(
                                psb[g % 4][:, 0:T],
                                wd0_sb[:, (i * 2 + j) * 128:(i * 2 + j) * 128 + 128],
                                cat_sb[:, i * T:(i + 1) * T],
                                start=(i == 0), stop=(i == NDK - 1))
                        thr[f"mm_g{g}"] = s_mm.inc(mm)
                        g += 1

                    for m in range(N_DEC - 2):
                        tensor.wait_ge(s_ld.h, thr["ld_wdec"])
                        tensor.wait_ge(s_st.h, st_zf(m))
                        for j in range(2):
                            if g >= 4:
                                tensor.wait_ge(s_act.h, act_g_thr(g - 4))
                            for i in range(NHD):
                                mm = tensor.matmul(
                                    psb[g % 4][:, 0:T],
                                    wdm_sb[m][:, (i * 2 + j) * 128:(i * 2 + j) * 128 + 128],
                                    zf_sb[:, i * T:(i + 1) * T],
                                    start=(i == 0), stop=(i == NHD - 1))
                            thr[f"mm_g{g}"] = s_mm.inc(mm)
                            g += 1

                    tensor.wait_ge(s_st.h, st_zf(N_DEC - 2))
                    tensor.wait_ge(s_act.h, act_g_thr(g - 4))
                    for i in range(NHD):
                        mm = tensor.matmul(
                            psb[g % 4][:, 0:T],
                            wdo_sb[:, i * 128:(i + 1) * 128],
                            zf_sb[:, i * T:(i + 1) * T],
                            start=(i == 0), stop=(i == NHD - 1))
                    thr["mm_y"] = s_mm.inc(mm)
                    thr["y_g"] = g
                    g += 1

            @block.scalar
            def _(scalar):
                for l in range(L):
                    for t in range(T):
                        for k in range(E_LOC):
                            if t == 0:
                                scalar.wait_ge(s_mm.h, thr[f"mm_u_{k}_{l}"])
                            else:
                                scalar.wait_ge(s_mm.h, thr[f"mm_rec_{k}_{l}_{t}"])
                            a = scalar.activation(
                                hb[k][:, l * DT * T + t::T][:, 0:DT],
                                pu[k][:, t::T],
                                AF.Tanh)
                            s_act.inc(a)

                gg = 0
                for k in range(E_LOC):
                    for j in range(NFT):
                        scalar.wait_ge(s_mm.h, thr[f"mm_g{gg}"])
                        a = scalar.activation(
                            ffs[k][:, j * T:(j + 1) * T], psb[gg % 4][:, 0:T],
                            AF.Gelu_apprx_tanh, bias=bff1_ap(k, j))
                        s_act.inc(a)
                        gg += 1

                for k in range(E_LOC):
                    for j in range(DT):
                        scalar.wait_ge(s_mm.h, thr[f"mm_g{gg}"])
                        a = scalar.activation(
                            enc_sb[k][:, j * T:(j + 1) * T], psb[gg % 4][:, 0:T],
                            AF.Identity, bias=bff2_ap(k, j))
                        thr[f"act_enc_{k}_{j}"] = s_act.inc(a)
                        gg += 1

                for j in range(2):
                    scalar.wait_ge(s_mm.h, thr[f"mm_g{gg}"])
                    a = scalar.activation(
                        zloc_sb[:, j * T:(j + 1) * T], psb[gg % 4][:, 0:T],
                        AF.Tanh, bias=bd0_ap(j))
                    thr[f"act_z0_{j}"] = s_act.inc(a)
                    gg += 1

                for m in range(N_DEC - 2):
                    for j in range(2):
                        scalar.wait_ge(s_mm.h, thr[f"mm_g{gg}"])
                        a = scalar.activation(
                            zloc_sb[:, j * T:(j + 1) * T], psb[gg % 4][:, 0:T],
                            AF.Tanh, bias=bdm_ap(m, j))
                        thr[f"act_zm_{m}_{j}"] = s_act.inc(a)
                        gg += 1

                scalar.wait_ge(s_mm.h, thr["mm_y"])
                a = scalar.activation(ysb[:, :], psb[thr["y_g"] % 4][:, 0:T],
                                      AF.Identity, bias=bdo_ap())
                thr["act_y"] = s_act.inc(a)

            @block.gpsimd
            def _(gpsimd):
                if debug:
                    gpsimd.wait_ge(s_ld.h, thr["ld_wh1"])
                    s_dbg.inc(gpsimd.dma_start(whd[:, :], wh_sb[0][:, :]), 16)
                for k in range(E_LOC):
                    gpsimd.wait_ge(s_act.h, thr[f"act_enc_{k}_{DT - 1}"])
                    d = gpsimd.dma_start(
                        ag0_in[k * D_ENC:(k + 1) * D_ENC, :].rearrange(
                            "(j p) t -> p j t", p=128),
                        enc_sb[k][:, :])
                    thr[f"st_enc{k}"] = s_st.inc(d, 16)
                gpsimd.wait_ge(s_st.h, thr[f"st_enc{E_LOC - 1}"])
                cc = gpsimd.collective_compute(
                    "AllGather", BYPASS, replica_groups=RG,
                    ins=[ag0_in[:, :]], outs=[ag0_out[:, :]])
                s_cc.inc(cc)
                gpsimd.wait_ge(s_cc.h, 1)
                d = gpsimd.dma_start(
                    cat_sb[:, :],
                    ag0_out[:, :].rearrange("(i p) t -> p i t", p=128))
                s_st.inc(d, 16)
                assert s_st.v == ST_CAT
                for m in range(N_DEC - 1):
                    if m == 0:
                        gpsimd.wait_ge(s_act.h, thr["act_z0_1"])
                    else:
                        gpsimd.wait_ge(s_act.h, thr[f"act_zm_{m - 1}_1"])
                    d = gpsimd.dma_start(
                        agz_in[m][:, :].rearrange("(j p) t -> p j t", p=128),
                        zloc_sb[:, :])
                    thr[f"st_z{m}"] = s_st.inc(d, 16)
                    assert thr[f"st_z{m}"] == st_z(m)
                    gpsimd.wait_ge(s_st.h, thr[f"st_z{m}"])
                    cc = gpsimd.collective_compute(
                        "AllGather", BYPASS, replica_groups=RG,
                        ins=[agz_in[m][:, :]], outs=[agz_out[m][:, :]])
                    s_cc.inc(cc)
                    gpsimd.wait_ge(s_cc.h, 1 + m + 1)
                    d = gpsimd.dma_start(
                        zf_sb[:, :],
                        agz_out[m][:, :].rearrange("(i p) t -> p i t", p=128))
                    s_st.inc(d, 16)
                    assert s_st.v == st_zf(m)
                gpsimd.wait_ge(s_act.h, thr["act_y"])
                d = gpsimd.dma_start(y_out[:, :], ysb[:, :])
                s_st.inc(d, 16)
                if debug:
                    for k in range(E_LOC):
                        s_dbg.inc(gpsimd.dma_start(hd[k][:, :], hb[k][:, :]), 16)
                        s_dbg.inc(gpsimd.dma_start(ed[k][:, :], enc_sb[k][:, :]), 16)
                        s_dbg.inc(gpsimd.dma_start(ffd[k][:, :], ffs[k][:, :]), 16)
                    s_dbg.inc(gpsimd.dma_start(catd[:, :], cat_sb[:, :]), 16)
                    s_dbg.inc(gpsimd.dma_start(zd[:, :], zloc_sb[:, :]), 16)
                    s_dbg.inc(gpsimd.dma_start(w0d[:, :], win0_sb[0][:, :]), 16)
                    s_dbg.inc(gpsimd.dma_start(xd[:, :], xT_sb[:, :]), 16)
                    gpsimd.wait_ge(s_dbg.h, s_dbg.v)
                gpsimd.wait_ge(s_st.h, s_st.v)  # drain final store

        nc.compile()
    return nc


def prep_inputs(inputs):
    """Build the 8 per-core input maps from full numpy inputs (all fp16)."""
    f32 = lambda a: np.asarray(a, np.float32)
    F16 = np.float16
    x = f32(inputs["x"])
    W_in0, Wh0, b0 = f32(inputs["W_in0"]), f32(inputs["Wh0"]), f32(inputs["b0"])
    W_in_rest, Wh_rest, b_rest = (f32(inputs["W_in_rest"]), f32(inputs["Wh_rest"]),
                                  f32(inputs["b_rest"]))
    W_ff1, b_ff1 = f32(inputs["W_ff1"]), f32(inputs["b_ff1"])
    W_ff2, b_ff2 = f32(inputs["W_ff2"]), f32(inputs["b_ff2"])
    W_d0, b_d0 = f32(inputs["W_d0"]), f32(inputs["b_d0"])
    W_dmid, b_dmid = f32(inputs["W_dmid"]), f32(inputs["b_dmid"])
    W_dout, b_dout = f32(inputs["W_dout"]), f32(inputs["b_dout"])

    xT_aug = np.concatenate([x[0].T, np.ones((1, T), np.float32)], 0)  # [33, T]
    in_maps = []
    for c in range(N_CORES):
        m = {"xT_aug": xT_aug.astype(F16),
             "ones_row": np.ones((1, T), F16)}
        for k in range(E_LOC):
            e = E_LOC * c + k
            m[f"win0_{k}"] = np.concatenate(
                [W_in0[e], b0[e][None, :]], 0).astype(F16)       # [33, 512]
            wh_all = np.concatenate([Wh0[e][None], Wh_rest[e]], 0)  # [3, D, D]
            m[f"wh_{k}"] = np.concatenate(
                [_tile_kxm(wh_all[l]) for l in range(L)], axis=1).astype(F16)
            m[f"win_{k}"] = np.concatenate(
                [_tile_kxm(W_in_rest[e][l]) for l in range(L - 1)], axis=1).astype(F16)
            m[f"brow_{k}"] = b_rest[e].reshape(1, (L - 1) * D).astype(F16)
            m[f"wff1_{k}"] = _tile_kxm(W_ff1[e]).astype(F16)
            m[f"wff2_{k}"] = _tile_kxm(W_ff2[e]).astype(F16)
        bc = np.zeros((128, NBC), np.float32)
        for k in range(E_LOC):
            e = E_LOC * c + k
            bc[:, k * NFT:(k + 1) * NFT] = _bias_cols(b_ff1[e])
            bc[:, 2 * NFT + k * DT:2 * NFT + (k + 1) * DT] = _bias_cols(b_ff2[e])
        o = 2 * NFT + 2 * DT
        bc[:, o:o + 2] = _bias_cols(b_d0[c * HD_SH:(c + 1) * HD_SH])
        for mm_ in range(N_DEC - 2):
            bc[:, o + 2 + 2 * mm_:o + 2 + 2 * (mm_ + 1)] = _bias_cols(
                b_dmid[mm_][c * HD_SH:(c + 1) * HD_SH])
        bc[:, o + 6:o + 7] = _bias_cols(b_dout[c * DO_SH:(c + 1) * DO_SH])
        m["bcols"] = bc
        m["wd0"] = _tile_kxm(W_d0[:, c * HD_SH:(c + 1) * HD_SH]).astype(F16)
        for mm_ in range(N_DEC - 2):
            m[f"wdm{mm_}"] = _tile_kxm(W_dmid[mm_][:, c * HD_SH:(c + 1) * HD_SH]).astype(F16)
        m["wdo"] = _tile_kxm(W_dout[:, c * DO_SH:(c + 1) * DO_SH]).astype(F16)
        in_maps.append(m)
    return in_maps


def run(inputs, t_steps=T, trace=False, debug=False):
    from concourse.bass_utils import run_bass_kernel_spmd

    nc = build_nc(debug=debug)
    in_maps = prep_inputs(inputs)
    res = run_bass_kernel_spmd(nc, in_maps, list(range(N_CORES)), trace=trace)
    parts = [res.results[c]["y_out"] for c in range(N_CORES)]  # each [128, T]
    y = np.concatenate([np.asarray(p, np.float32).T for p in parts], axis=1)
    return y[None], res


def kernel(**inputs):
    y, _ = run(inputs, T, trace=False)
    return y


# revision 21
# speedup vs baseline: 1.9555x; 1.0074x over previous
"""HRNN Trainium2 kernel v2: 16 encoders (3-layer tanh RNN + FF) -> 4-layer decoder.

Sharding: expert-parallel, 2 encoders/core over 8 cores; decoder column-sharded
(256 of H_DEC, 128 of D_OUT per core) with AllGathers between layers.

Raw-bass implementation (no tile framework): fp16 weights/states throughout,
u-contributions pre-accumulated into PSUM banks so each recurrence step is
16 matmuls (N=1) + 1 strided tanh ACT with a single semaphore pair per
(encoder, step). Batched matmuls (u-precompute, FF1/FF2, decoder) accumulate
N=128 groups in rotating PSUM tiles. Weight DMA, collectives, and mid-kernel
stores overlap compute via dedicated engine streams. Decoder weights reuse the
recurrence-weight SBUF region (semaphore-gated); decoder activations reuse the
FF1-weight region (collective-chain gated).
"""

import sys
import numpy as np

sys.path.insert(0, "/opt/trn_rl_repo")

E = 16
L = 3
D_IN = 32
D = 512
H_FF = 2048
D_ENC = 512
N_DEC = 4
H_DEC = 2048
D_OUT = 1024
T = 128
N_CORES = 8

E_LOC = E // N_CORES          # 2 encoders per core
DT = D // 128                 # 4 d-tiles
HD_SH = H_DEC // N_CORES      # 256 decoder hidden per core
DO_SH = D_OUT // N_CORES      # 128 output dims per core
NFT = H_FF // 128             # 16 ff tiles
NCAT = (L * D) // 128         # 12 cat tiles
NDK = (E * D_ENC) // 128      # 64 decoder-input k-tiles
NHD = H_DEC // 128            # 16
NBC = 2 * NFT + 2 * DT + 2 + 4 + 1  # packed bias cols


def _tile_kxm(w):
    """[K, M] -> [128, nk*nm*128] with col ((i*nm)+j)*128 : lhsT tile (i,j)."""
    K, M = w.shape
    nk, nm = K // 128, M // 128
    return np.ascontiguousarray(
        w.reshape(nk, 128, nm, 128).transpose(1, 0, 2, 3).reshape(128, nk * nm * 128)
    )


def _bias_cols(b):
    """[M] -> [128, M//128] with col j holding b[j*128:(j+1)*128]."""
    return np.ascontiguousarray(b.reshape(-1, 128).T)


class Ctr:
    """Python-side absolute counter for a hardware semaphore."""

    def __init__(self, handle):
        self.h = handle
        self.v = 0

    def inc(self, inst, amt=1):
        inst.then_inc(self.h, amt)
        self.v += amt
        return self.v


# ---- semaphore threshold formulas (must match emission order) -------------
def act_rec_thr(k, l, t):
    # scalar inc order: for l: for t: for k
    return l * T * E_LOC + t * E_LOC + k + 1


def act_g_thr(gg):
    # batched-group ACTs follow all recurrence ACTs, in group order
    return E_LOC * L * T + gg + 1


ST_ENC = [16, 32]
ST_CAT = [48, 64]


def st_z(m):
    return 80 + 32 * m


def st_zf(m):
    return 96 + 32 * m


ST_Y = 80 + 32 * (N_DEC - 1)


def build_nc(debug=False):
    from contextlib import ExitStack

    from concourse import bacc, mybir

    F32 = mybir.dt.float32
    FP16 = mybir.dt.float16
    U8 = mybir.dt.uint8
    AF = mybir.ActivationFunctionType
    BYPASS = mybir.AluOpType.bypass

    nc = bacc.Bacc(None, num_devices=N_CORES)

    # ---- DRAM I/O ---------------------------------------------------------
    xT_aug = nc.dram_tensor("xT_aug", [D_IN + 1, T], FP16, kind="ExternalInput")
    ones_row = nc.dram_tensor("ones_row", [1, T], FP16, kind="ExternalInput")
    win0 = [nc.dram_tensor(f"win0_{k}", [D_IN + 1, D], FP16, kind="ExternalInput")
            for k in range(E_LOC)]
    wh = [nc.dram_tensor(f"wh_{k}", [128, L * 16 * 128], FP16, kind="ExternalInput")
          for k in range(E_LOC)]
    win = [nc.dram_tensor(f"win_{k}", [128, (L - 1) * 16 * 128], FP16, kind="ExternalInput")
           for k in range(E_LOC)]
    brow = [nc.dram_tensor(f"brow_{k}", [1, (L - 1) * D], FP16, kind="ExternalInput")
            for k in range(E_LOC)]
    wff1 = [nc.dram_tensor(f"wff1_{k}", [128, NCAT * NFT * 128], FP16, kind="ExternalInput")
            for k in range(E_LOC)]
    wff2 = [nc.dram_tensor(f"wff2_{k}", [128, NFT * DT * 128], FP16, kind="ExternalInput")
            for k in range(E_LOC)]
    bcols = nc.dram_tensor("bcols", [128, NBC], F32, kind="ExternalInput")
    wd0 = nc.dram_tensor("wd0", [128, NDK * 2 * 128], FP16, kind="ExternalInput")
    wdm = [nc.dram_tensor(f"wdm{m}", [128, NHD * 2 * 128], FP16, kind="ExternalInput")
           for m in range(N_DEC - 2)]
    wdo = nc.dram_tensor("wdo", [128, NHD * 128], FP16, kind="ExternalInput")
    y_out = nc.dram_tensor("y_out", [DO_SH, T], F32, kind="ExternalOutput")
    if debug:
        hd = [nc.dram_tensor(f"hd{k}", [128, L * DT * T], FP16, kind="ExternalOutput")
              for k in range(E_LOC)]
        ed = [nc.dram_tensor(f"ed{k}", [128, DT * T], FP16, kind="ExternalOutput")
              for k in range(E_LOC)]
        ffd = [nc.dram_tensor(f"ffd{k}", [128, NFT * T], FP16, kind="ExternalOutput")
               for k in range(E_LOC)]
        zd = nc.dram_tensor("zd", [128, 2 * T], FP16, kind="ExternalOutput")
        whd = nc.dram_tensor("whd", [128, L * 16 * 128], FP16, kind="ExternalOutput")
        w0d = nc.dram_tensor("w0d", [D_IN + 1, D], FP16, kind="ExternalOutput")
        xd = nc.dram_tensor("xd", [D_IN + 1, T], FP16, kind="ExternalOutput")

    ag0_in = [nc.dram_tensor(f"ag0_in{k}", [D_ENC, T], FP16) for k in range(E_LOC)]
    ag0_out = [nc.dram_tensor(f"ag0_out{k}", [N_CORES * D_ENC, T], FP16, addr_space="Shared")
               for k in range(E_LOC)]
    agz_in = [nc.dram_tensor(f"agz_in{m}", [HD_SH, T], FP16) for m in range(N_DEC - 1)]
    agz_out = [nc.dram_tensor(f"agz_out{m}", [H_DEC, T], FP16, addr_space="Shared")
               for m in range(N_DEC - 1)]
    RG = [list(range(N_CORES))]

    def whcol(l, i, j):
        return ((l * 16) + i * 4 + j) * 128

    def wincol(l, i, j):
        return ((l - 1) * 16 * 128) + (i * 4 + j) * 128

    with ExitStack() as ctx:
        ec = ctx.enter_context

        s_ld = Ctr(ec(nc.semaphore("s_ld")))
        s_mm = Ctr(ec(nc.semaphore("s_mm")))
        s_act = Ctr(ec(nc.semaphore("s_act")))
        s_st = Ctr(ec(nc.semaphore("s_st")))
        s_cc = Ctr(ec(nc.semaphore("s_cc")))
        s_dbg = Ctr(ec(nc.semaphore("s_dbg")))

        # ---- SBUF: left = long-lived ------------------------------------
        xT_sb = ec(nc.sbuf_tensor("xT_sb", [D_IN + 1, T], FP16))
        ones_sb = ec(nc.sbuf_tensor("ones_sb", [1, T], FP16))
        win0_sb = [ec(nc.sbuf_tensor(f"win0_sb{k}", [D_IN + 1, D], FP16))
                   for k in range(E_LOC)]
        brow_sb = [ec(nc.sbuf_tensor(f"brow_sb{k}", [1, (L - 1) * D], FP16))
                   for k in range(E_LOC)]
        bc_sb = ec(nc.sbuf_tensor("bc_sb", [128, NBC], F32))
        hb = [ec(nc.sbuf_tensor(f"hb{k}", [128, L * DT * T], FP16))
              for k in range(E_LOC)]
        ffs = [ec(nc.sbuf_tensor(f"ffs{k}", [128, NFT * T], FP16))
               for k in range(E_LOC)]
        enc_sb = [ec(nc.sbuf_tensor(f"enc{k}", [128, DT * T], FP16))
                  for k in range(E_LOC)]
        wff1_sb = [ec(nc.sbuf_tensor(f"wff1_sb{k}", [128, NCAT * NFT * 128], FP16))
                   for k in range(E_LOC)]
        wff2_sb = [ec(nc.sbuf_tensor(f"wff2_sb{k}", [128, NFT * DT * 128], FP16))
                   for k in range(E_LOC)]

        # ---- SBUF: right = 52KB arena, phase-aliased --------------------
        ARENA = 52 * 1024
        arena = ec(nc.sbuf_tensor("arena", [128, ARENA], U8, side="right"))
        abase = nc.lookup_mloc(arena).addr

        def at(name, shape, dtype, off):
            return nc.alloc_sbuf_tensor_at(name, shape, dtype, offset=abase + off)

        # phase 1 (recurrence): wh (12KB x2) + win (8KB x2) = 40KB
        wh_sb = [at(f"wh_sb{k}", [128, L * 16 * 128], FP16, k * 12 * 1024)
                 for k in range(E_LOC)]
        win_sb = [at(f"win_sb{k}", [128, (L - 1) * 16 * 128], FP16,
                     24 * 1024 + k * 8 * 1024) for k in range(E_LOC)]
        # phase 2 (decoder weights; DMA gated on recurrence completion)
        wd0_sb = at("wd0_sb", [128, NDK * 2 * 128], FP16, 0)            # 32KB
        wdm_sb = [at(f"wdm_sb{m}", [128, NHD * 2 * 128], FP16,
                     32 * 1024 + m * 8 * 1024) for m in range(N_DEC - 2)]  # 8KB x2
        wdo_sb = at("wdo_sb", [128, NHD * 128], FP16, 48 * 1024)        # 4KB

        # decoder activations alias the FF1[0] weight region (dead after FF1)
        wbase = nc.lookup_mloc(wff1_sb[0]).addr

        def at_w(name, shape, dtype, off):
            return nc.alloc_sbuf_tensor_at(name, shape, dtype, offset=wbase + off)

        cat_sb = [at_w(f"cat_sb{k}", [128, NDK // 2 * T], FP16, k * 8 * 1024)
                  for k in range(E_LOC)]                                  # 8KB x2
        zf_sb = at_w("zf_sb", [128, NHD * T], FP16, 16 * 1024)          # 4KB
        zloc_sb = at_w("zloc_sb", [128, 2 * T], FP16, 20 * 1024)        # 512B
        ysb = at_w("ysb", [DO_SH, T], F32, 21 * 1024)                   # 512B

        # ---- PSUM -------------------------------------------------------
        pu = [ec(nc.psum_tensor(f"pu{k}", [128, DT * T], F32)) for k in range(E_LOC)]
        psb = [ec(nc.psum_tensor(f"psb{g}", [128, 512], F32)) for g in range(4)]

        def bff1_ap(k, j):
            return bc_sb[:, k * NFT + j:k * NFT + j + 1]

        def bff2_ap(k, j):
            c = 2 * NFT + k * DT + j
            return bc_sb[:, c:c + 1]

        def bd0_ap(j):
            c = 2 * NFT + 2 * DT + j
            return bc_sb[:, c:c + 1]

        def bdm_ap(m, j):
            c = 2 * NFT + 2 * DT + 2 + m * 2 + j
            return bc_sb[:, c:c + 1]

        def bdo_ap():
            c = 2 * NFT + 2 * DT + 2 + 4
            return bc_sb[:, c:c + 1]

        thr = {}

        with nc.Block() as block:

            @block.sync
            def _(sync):
                def load(dst, src):
                    return s_ld.inc(sync.dma_start(dst, src), 16)

                load(xT_sb[:, :], xT_aug[:, :])
                load(ones_sb[:, :], ones_row[:, :])
                load(bc_sb[:, :], bcols[:, :])
                for k in range(E_LOC):
                    load(win0_sb[k][:, :], win0[k][:, :])
                    load(brow_sb[k][:, :], brow[k][:, :])
                for k in range(E_LOC):
                    thr[f"ld_whl0_{k}"] = load(wh_sb[k][:, 0:16 * 128],
                                               wh[k][:, 0:16 * 128])
                for k in range(E_LOC):
                    thr[f"ld_win{k}"] = load(win_sb[k][:, :], win[k][:, :])
                for l in (1, 2):
                    for k in range(E_LOC):
                        thr[f"ld_whl{l}_{k}"] = load(
                            wh_sb[k][:, l * 16 * 128:(l + 1) * 16 * 128],
                            wh[k][:, l * 16 * 128:(l + 1) * 16 * 128])
                for k in range(E_LOC):
                    thr[f"ld_wff1{k}"] = load(wff1_sb[k][:, :], wff1[k][:, :])
                for k in range(E_LOC):
                    thr[f"ld_wff2{k}"] = load(wff2_sb[k][:, :], wff2[k][:, :])
                # decoder weights alias wh/win: wait until the recurrence is done
                sync.wait_ge(s_act.h, E_LOC * L * T)
                thr["ld_wd0"] = load(wd0_sb[:, :], wd0[:, :])
                for m in range(N_DEC - 2):
                    load(wdm_sb[m][:, :], wdm[m][:, :])
                thr["ld_wdec"] = load(wdo_sb[:, :], wdo[:, :])

            @block.tensor
            def _(tensor):
                for l in range(L):
                    tensor.wait_ge(s_ld.h, thr[f"ld_whl{l}_1"])
                    for k in range(E_LOC):
                        if l == 0:
                            for j in range(DT):
                                mm = tensor.matmul(
                                    pu[k][:, j * T:(j + 1) * T],
                                    win0_sb[k][:, j * 128:(j + 1) * 128],
                                    xT_sb[:, :],
                                    start=(j == 0), stop=True,
                                    skip_group_check=True)
                        else:
                            if k == 0 and l == 1:
                                tensor.wait_ge(s_ld.h, thr["ld_win1"])
                            tensor.wait_ge(s_act.h, act_rec_thr(k, l - 1, T - 1))
                            for j in range(DT):
                                for i in range(DT):
                                    tensor.matmul(
                                        pu[k][:, j * T:(j + 1) * T],
                                        win_sb[k][:, wincol(l, i, j):wincol(l, i, j) + 128],
                                        hb[k][:, ((l - 1) * DT + i) * T:((l - 1) * DT + i + 1) * T],
                                        start=(j == 0 and i == 0), stop=False,
                                        skip_group_check=True)
                                mm = tensor.matmul(
                                    pu[k][:, j * T:(j + 1) * T],
                                    brow_sb[k][0:1, (l - 1) * D + j * 128:(l - 1) * D + (j + 1) * 128],
                                    ones_sb[0:1, :],
                                    start=False, stop=True,
                                    skip_group_check=True)
                        thr[f"mm_u_{k}_{l}"] = s_mm.inc(mm)

                    with nc.named_scope(f"rec{l}"):
                        for t in range(1, T):
                            for k in range(E_LOC):
                                tensor.wait_ge(s_act.h, act_rec_thr(k, l, t - 1))
                                for j in range(DT):
                                    for i in range(DT):
                                        mm = tensor.matmul(
                                            pu[k][:, j * T + t:j * T + t + 1],
                                            wh_sb[k][:, whcol(l, i, j):whcol(l, i, j) + 128],
                                            hb[k][:, (l * DT + i) * T + t - 1:(l * DT + i) * T + t],
                                            start=False, stop=(i == DT - 1),
                                            skip_group_check=True)
                                thr[f"mm_rec_{k}_{l}_{t}"] = s_mm.inc(mm)

                # --- FF1 -------------------------------------------------
                g = 0
                tensor.wait_ge(s_ld.h, thr["ld_wff11"])
                with nc.named_scope("ff1"):
                    for k in range(E_LOC):
                        tensor.wait_ge(s_act.h, act_rec_thr(k, L - 1, T - 1))
                        for j in range(NFT):
                            if g >= 4:
                                tensor.wait_ge(s_act.h, act_g_thr(g - 4))
                            for i in range(NCAT):
                                mm = tensor.matmul(
                                    psb[g % 4][:, 0:T],
                                    wff1_sb[k][:, (i * NFT + j) * 128:(i * NFT + j) * 128 + 128],
                                    hb[k][:, i * T:(i + 1) * T],
                                    start=(i == 0), stop=(i == NCAT - 1))
                            thr[f"mm_g{g}"] = s_mm.inc(mm)
                            g += 1

                # --- FF2 -------------------------------------------------
                tensor.wait_ge(s_ld.h, thr["ld_wff21"])
                with nc.named_scope("ff2"):
                    for k in range(E_LOC):
                        tensor.wait_ge(s_act.h, act_g_thr(k * NFT + NFT - 1))
                        for j in range(DT):
                            if g >= 4:
                                tensor.wait_ge(s_act.h, act_g_thr(g - 4))
                            for i in range(NFT):
                                mm = tensor.matmul(
                                    psb[g % 4][:, 0:T],
                                    wff2_sb[k][:, (i * DT + j) * 128:(i * DT + j) * 128 + 128],
                                    ffs[k][:, i * T:(i + 1) * T],
                                    start=(i == 0), stop=(i == NFT - 1))
                            thr[f"mm_g{g}"] = s_mm.inc(mm)
                            g += 1

                # --- decoder ---------------------------------------------
                tensor.wait_ge(s_ld.h, thr["ld_wd0"])
                tensor.wait_ge(s_st.h, ST_CAT[1])
                with nc.named_scope("dec"):
                    for j in range(2):
                        if g >= 4:
                            tensor.wait_ge(s_act.h, act_g_thr(g - 4))
                        for i in range(NDK):
                            e_, dt_ = i // DT, i % DT
                            kk, cc_ = e_ % 2, e_ // 2
                            mm = tensor.matmul(
                                psb[g % 4][:, 0:T],
                                wd0_sb[:, (i * 2 + j) * 128:(i * 2 + j) * 128 + 128],
                                cat_sb[kk][:, (cc_ * DT + dt_) * T:(cc_ * DT + dt_ + 1) * T],
                                start=(i == 0), stop=(i == NDK - 1))
                        thr[f"mm_g{g}"] = s_mm.inc(mm)
                        g += 1

                    for m in range(N_DEC - 2):
                        tensor.wait_ge(s_ld.h, thr["ld_wdec"])
                        tensor.wait_ge(s_st.h, st_zf(m))
                        for j in range(2):
                            if g >= 4:
                                tensor.wait_ge(s_act.h, act_g_thr(g - 4))
                            for i in range(NHD):
                                mm = tensor.matmul(
                                    psb[g % 4][:, 0:T],
                                    wdm_sb[m][:, (i * 2 + j) * 128:(i * 2 + j) * 128 + 128],
                                    zf_sb[:, i * T:(i + 1) * T],
                                    start=(i == 0), stop=(i == NHD - 1))
                            thr[f"mm_g{g}"] = s_mm.inc(mm)
                            g += 1

                    tensor.wait_ge(s_st.h, st_zf(N_DEC - 2))
                    tensor.wait_ge(s_act.h, act_g_thr(g - 4))
                    for i in range(NHD):
                        mm = tensor.matmul(
                            psb[g % 4][:, 0:T],
                            wdo_sb[:, i * 128:(i + 1) * 128],
                            zf_sb[:, i * T:(i + 1) * T],
                            start=(i == 0), stop=(i == NHD - 1))
                    thr["mm_y"] = s_mm.inc(mm)
                    thr["y_g"] = g
                    g += 1

            @block.scalar
            def _(scalar):
                for l in range(L):
                    for t in range(T):
                        for k in range(E_LOC):
                            if t == 0:
                                scalar.wait_ge(s_mm.h, thr[f"mm_u_{k}_{l}"])
                            else:
                                scalar.wait_ge(s_mm.h, thr[f"mm_rec_{k}_{l}_{t}"])
                            a = scalar.activation(
                                hb[k][:, l * DT * T + t::T][:, 0:DT],
                                pu[k][:, t::T],
                                AF.Tanh)
                            s_act.inc(a)

                gg = 0
                for k in range(E_LOC):
                    for j in range(NFT):
                        scalar.wait_ge(s_mm.h, thr[f"mm_g{gg}"])
                        a = scalar.activation(
                            ffs[k][:, j * T:(j + 1) * T], psb[gg % 4][:, 0:T],
                            AF.Gelu_apprx_tanh, bias=bff1_ap(k, j))
                        s_act.inc(a)
                        gg += 1

                for k in range(E_LOC):
                    for j in range(DT):
                        scalar.wait_ge(s_mm.h, thr[f"mm_g{gg}"])
                        a = scalar.activation(
                            enc_sb[k][:, j * T:(j + 1) * T], psb[gg % 4][:, 0:T],
                            AF.Identity, bias=bff2_ap(k, j))
                        thr[f"act_enc_{k}_{j}"] = s_act.inc(a)
                        gg += 1
                    # store this encoder's output for its AllGather
                    scalar.wait_ge(s_act.h, thr[f"act_enc_{k}_{DT - 1}"])
                    d = scalar.dma_start(
                        ag0_in[k][:, :].rearrange("(j p) t -> p j t", p=128),
                        enc_sb[k][:, :])
                    thr[f"st_enc{k}"] = s_st.inc(d, 16)
                    assert thr[f"st_enc{k}"] == ST_ENC[k]

                for k in range(E_LOC):
                    scalar.wait_ge(s_cc.h, k + 1)
                    d = scalar.dma_start(
                        cat_sb[k][:, :],
                        ag0_out[k][:, :].rearrange("(i p) t -> p i t", p=128))
                    s_st.inc(d, 16)
                    assert s_st.v == ST_CAT[k]

                for j in range(2):
                    scalar.wait_ge(s_mm.h, thr[f"mm_g{gg}"])
                    a = scalar.activation(
                        zloc_sb[:, j * T:(j + 1) * T], psb[gg % 4][:, 0:T],
                        AF.Tanh, bias=bd0_ap(j))
                    thr[f"act_z0_{j}"] = s_act.inc(a)
                    gg += 1
                scalar.wait_ge(s_act.h, thr["act_z0_1"])
                d = scalar.dma_start(
                    agz_in[0][:, :].rearrange("(j p) t -> p j t", p=128),
                    zloc_sb[:, :])
                thr["st_z0"] = s_st.inc(d, 16)
                assert thr["st_z0"] == st_z(0)

                for m in range(N_DEC - 2):
                    scalar.wait_ge(s_cc.h, E_LOC + m + 1)
                    d = scalar.dma_start(
                        zf_sb[:, :],
                        agz_out[m][:, :].rearrange("(i p) t -> p i t", p=128))
                    s_st.inc(d, 16)
                    assert s_st.v == st_zf(m)
                    for j in range(2):
                        scalar.wait_ge(s_mm.h, thr[f"mm_g{gg}"])
                        a = scalar.activation(
                            zloc_sb[:, j * T:(j + 1) * T], psb[gg % 4][:, 0:T],
                            AF.Tanh, bias=bdm_ap(m, j))
                        thr[f"act_zm_{m}_{j}"] = s_act.inc(a)
                        gg += 1
                    scalar.wait_ge(s_act.h, thr[f"act_zm_{m}_1"])
                    d = scalar.dma_start(
                        agz_in[m + 1][:, :].rearrange("(j p) t -> p j t", p=128),
                        zloc_sb[:, :])
                    thr[f"st_z{m + 1}"] = s_st.inc(d, 16)
                    assert thr[f"st_z{m + 1}"] == st_z(m + 1)

                scalar.wait_ge(s_cc.h, E_LOC + N_DEC - 1)
                d = scalar.dma_start(
                    zf_sb[:, :],
                    agz_out[N_DEC - 2][:, :].rearrange("(i p) t -> p i t", p=128))
                s_st.inc(d, 16)
                assert s_st.v == st_zf(N_DEC - 2)

                scalar.wait_ge(s_mm.h, thr["mm_y"])
                a = scalar.activation(ysb[:, :], psb[thr["y_g"] % 4][:, 0:T],
                                      AF.Identity, bias=bdo_ap())
                thr["act_y"] = s_act.inc(a)
                scalar.wait_ge(s_act.h, thr["act_y"])
                d = scalar.dma_start(y_out[:, :], ysb[:, :])
                s_st.inc(d, 16)
                scalar.wait_ge(s_st.h, s_st.v)  # drain final store

            @block.gpsimd
            def _(gpsimd):
                if debug:
                    gpsimd.wait_ge(s_ld.h, thr["ld_whl2_1"])
                    s_dbg.inc(gpsimd.dma_start(whd[:, :], wh_sb[0][:, :]), 16)
                for k in range(E_LOC):
                    gpsimd.wait_ge(s_st.h, thr[f"st_enc{k}"])
                    cc = gpsimd.collective_compute(
                        "AllGather", BYPASS, replica_groups=RG,
                        ins=[ag0_in[k][:, :]], outs=[ag0_out[k][:, :]])
                    s_cc.inc(cc)
                for m in range(N_DEC - 1):
                    gpsimd.wait_ge(s_st.h, thr[f"st_z{m}"])
                    cc = gpsimd.collective_compute(
                        "AllGather", BYPASS, replica_groups=RG,
                        ins=[agz_in[m][:, :]], outs=[agz_out[m][:, :]])
                    s_cc.inc(cc)
                if debug:
                    gpsimd.wait_ge(s_st.h, ST_Y + 16)
                    for k in range(E_LOC):
                        s_dbg.inc(gpsimd.dma_start(hd[k][:, :], hb[k][:, :]), 16)
                        s_dbg.inc(gpsimd.dma_start(ed[k][:, :], enc_sb[k][:, :]), 16)
                        s_dbg.inc(gpsimd.dma_start(ffd[k][:, :], ffs[k][:, :]), 16)
                    s_dbg.inc(gpsimd.dma_start(zd[:, :], zloc_sb[:, :]), 16)
                    s_dbg.inc(gpsimd.dma_start(w0d[:, :], win0_sb[0][:, :]), 16)
                    s_dbg.inc(gpsimd.dma_start(xd[:, :], xT_sb[:, :]), 16)
                    gpsimd.wait_ge(s_dbg.h, s_dbg.v)

        nc.compile()
    return nc


def prep_inputs(inputs):
    """Build the 8 per-core input maps from full numpy inputs (all fp16)."""
    f32 = lambda a: np.asarray(a, np.float32)
    F16 = np.float16
    x = f32(inputs["x"])
    W_in0, Wh0, b0 = f32(inputs["W_in0"]), f32(inputs["Wh0"]), f32(inputs["b0"])
    W_in_rest, Wh_rest, b_rest = (f32(inputs["W_in_rest"]), f32(inputs["Wh_rest"]),
                                  f32(inputs["b_rest"]))
    W_ff1, b_ff1 = f32(inputs["W_ff1"]), f32(inputs["b_ff1"])
    W_ff2, b_ff2 = f32(inputs["W_ff2"]), f32(inputs["b_ff2"])
    W_d0, b_d0 = f32(inputs["W_d0"]), f32(inputs["b_d0"])
    W_dmid, b_dmid = f32(inputs["W_dmid"]), f32(inputs["b_dmid"])
    W_dout, b_dout = f32(inputs["W_dout"]), f32(inputs["b_dout"])

    xT_aug = np.concatenate([x[0].T, np.ones((1, T), np.float32)], 0)  # [33, T]
    in_maps = []
    for c in range(N_CORES):
        m = {"xT_aug": xT_aug.astype(F16),
             "ones_row": np.ones((1, T), F16)}
        for k in range(E_LOC):
            e = E_LOC * c + k
            m[f"win0_{k}"] = np.concatenate(
                [W_in0[e], b0[e][None, :]], 0).astype(F16)       # [33, 512]
            wh_all = np.concatenate([Wh0[e][None], Wh_rest[e]], 0)  # [3, D, D]
            m[f"wh_{k}"] = np.concatenate(
                [_tile_kxm(wh_all[l]) for l in range(L)], axis=1).astype(F16)
            m[f"win_{k}"] = np.concatenate(
                [_tile_kxm(W_in_rest[e][l]) for l in range(L - 1)], axis=1).astype(F16)
            m[f"brow_{k}"] = b_rest[e].reshape(1, (L - 1) * D).astype(F16)
            m[f"wff1_{k}"] = _tile_kxm(W_ff1[e]).astype(F16)
            m[f"wff2_{k}"] = _tile_kxm(W_ff2[e]).astype(F16)
        bc = np.zeros((128, NBC), np.float32)
        for k in range(E_LOC):
            e = E_LOC * c + k
            bc[:, k * NFT:(k + 1) * NFT] = _bias_cols(b_ff1[e])
            bc[:, 2 * NFT + k * DT:2 * NFT + (k + 1) * DT] = _bias_cols(b_ff2[e])
        o = 2 * NFT + 2 * DT
        bc[:, o:o + 2] = _bias_cols(b_d0[c * HD_SH:(c + 1) * HD_SH])
        for mm_ in range(N_DEC - 2):
            bc[:, o + 2 + 2 * mm_:o + 2 + 2 * (mm_ + 1)] = _bias_cols(
                b_dmid[mm_][c * HD_SH:(c + 1) * HD_SH])
        bc[:, o + 6:o + 7] = _bias_cols(b_dout[c * DO_SH:(c + 1) * DO_SH])
        m["bcols"] = bc
        m["wd0"] = _tile_kxm(W_d0[:, c * HD_SH:(c + 1) * HD_SH]).astype(F16)
        for mm_ in range(N_DEC - 2):
            m[f"wdm{mm_}"] = _tile_kxm(W_dmid[mm_][:, c * HD_SH:(c + 1) * HD_SH]).astype(F16)
        m["wdo"] = _tile_kxm(W_dout[:, c * DO_SH:(c + 1) * DO_SH]).astype(F16)
        in_maps.append(m)
    return in_maps


def run(inputs, t_steps=T, trace=False, debug=False):
    from concourse.bass_utils import run_bass_kernel_spmd

    nc = build_nc(debug=debug)
    in_maps = prep_inputs(inputs)
    res = run_bass_kernel_spmd(nc, in_maps, list(range(N_CORES)), trace=trace)
    parts = [res.results[c]["y_out"] for c in range(N_CORES)]  # each [128, T]
    y = np.concatenate([np.asarray(p, np.float32).T for p in parts], axis=1)
    return y[None], res


def kernel(**inputs):
    y, _ = run(inputs, T, trace=False)
    return y


# revision 27
# speedup vs baseline: 1.9645x; 1.0046x over previous
"""HRNN Trainium2 kernel v2: 16 encoders (3-layer tanh RNN + FF) -> 4-layer decoder.

Sharding: expert-parallel, 2 encoders/core over 8 cores; decoder column-sharded
(256 of H_DEC, 128 of D_OUT per core) with AllGathers between layers.

Raw-bass implementation (no tile framework): fp16 weights/states throughout,
u-contributions pre-accumulated into PSUM banks so each recurrence step is
16 matmuls (N=1) + 1 strided tanh ACT with a single semaphore pair per
(encoder, step). Batched matmuls (u-precompute, FF1/FF2, decoder) accumulate
N=128 groups in rotating PSUM tiles. Weight DMA, collectives, and mid-kernel
stores overlap compute via dedicated engine streams. Decoder weights reuse the
recurrence-weight SBUF region (semaphore-gated); decoder activations reuse the
FF1-weight region (collective-chain gated).
"""

import sys
import numpy as np

sys.path.insert(0, "/opt/trn_rl_repo")

E = 16
L = 3
D_IN = 32
D = 512
H_FF = 2048
D_ENC = 512
N_DEC = 4
H_DEC = 2048
D_OUT = 1024
T = 128
N_CORES = 8

E_LOC = E // N_CORES          # 2 encoders per core
DT = D // 128                 # 4 d-tiles
HD_SH = H_DEC // N_CORES      # 256 decoder hidden per core
DO_SH = D_OUT // N_CORES      # 128 output dims per core
NFT = H_FF // 128             # 16 ff tiles
NCAT = (L * D) // 128         # 12 cat tiles
NDK = (E * D_ENC) // 128      # 64 decoder-input k-tiles
NHD = H_DEC // 128            # 16
NBC = 2 * NFT + 2 * DT + NHD + 2 * NHD + D_OUT // 128  # packed bias cols


def _tile_kxm(w):
    """[K, M] -> [128, nk*nm*128] with col ((i*nm)+j)*128 : lhsT tile (i,j)."""
    K, M = w.shape
    nk, nm = K // 128, M // 128
    return np.ascontiguousarray(
        w.reshape(nk, 128, nm, 128).transpose(1, 0, 2, 3).reshape(128, nk * nm * 128)
    )


def _bias_cols(b):
    """[M] -> [128, M//128] with col j holding b[j*128:(j+1)*128]."""
    return np.ascontiguousarray(b.reshape(-1, 128).T)


class Ctr:
    """Python-side absolute counter for a hardware semaphore."""

    def __init__(self, handle):
        self.h = handle
        self.v = 0

    def inc(self, inst, amt=1):
        inst.then_inc(self.h, amt)
        self.v += amt
        return self.v


# ---- semaphore threshold formulas (must match emission order) -------------
def act_rec_thr(k, l, t):
    # scalar inc order: for l: for t: for k
    return l * T * E_LOC + t * E_LOC + k + 1


def act_g_thr(gg):
    # batched-group ACTs follow all recurrence ACTs, in group order
    return E_LOC * L * T + gg + 1 + act_extra(gg)


def st_z(m):
    return 16 + 32 * m


def st_zf(m):
    return 32 + 32 * m


ST_ZY = 112
ST_Y = 128
N_REC_ACT = E_LOC * L * T


def act_extra(gg):
    # non-group s_act incs (post-RS tanh) interleaved before group gg
    if gg < 40 + NHD:
        return 0
    if gg < 40 + 2 * NHD:
        return 1
    if gg < 40 + 3 * NHD:
        return 2
    return 3


def th_tanh(m):
    # s_act value after the tanh of decoder shard m
    return N_REC_ACT + 40 + NHD + NHD * m + (m + 1)


def build_nc(debug=False):
    from contextlib import ExitStack

    from concourse import bacc, mybir

    F32 = mybir.dt.float32
    FP16 = mybir.dt.float16
    U8 = mybir.dt.uint8
    AF = mybir.ActivationFunctionType
    BYPASS = mybir.AluOpType.bypass

    nc = bacc.Bacc(None, num_devices=N_CORES)

    # ---- DRAM I/O ---------------------------------------------------------
    xT_aug = nc.dram_tensor("xT_aug", [D_IN + 1, T], FP16, kind="ExternalInput")
    ones_row = nc.dram_tensor("ones_row", [1, T], FP16, kind="ExternalInput")
    win0 = [nc.dram_tensor(f"win0_{k}", [D_IN + 1, D], FP16, kind="ExternalInput")
            for k in range(E_LOC)]
    wh = [nc.dram_tensor(f"wh_{k}", [128, L * 16 * 128], FP16, kind="ExternalInput")
          for k in range(E_LOC)]
    win = [nc.dram_tensor(f"win_{k}", [128, (L - 1) * 16 * 128], FP16, kind="ExternalInput")
           for k in range(E_LOC)]
    brow = [nc.dram_tensor(f"brow_{k}", [1, (L - 1) * D], FP16, kind="ExternalInput")
            for k in range(E_LOC)]
    wff1 = [nc.dram_tensor(f"wff1_{k}", [128, NCAT * NFT * 128], FP16, kind="ExternalInput")
            for k in range(E_LOC)]
    wff2 = [nc.dram_tensor(f"wff2_{k}", [128, NFT * DT * 128], FP16, kind="ExternalInput")
            for k in range(E_LOC)]
    bcols = nc.dram_tensor("bcols", [128, NBC], F32, kind="ExternalInput")
    wd0 = nc.dram_tensor("wd0", [128, NDK * 2 * 128], FP16, kind="ExternalInput")
    wdm = [nc.dram_tensor(f"wdm{m}", [128, NHD * 2 * 128], FP16, kind="ExternalInput")
           for m in range(N_DEC - 2)]
    wdo = nc.dram_tensor("wdo", [128, NHD * 128], FP16, kind="ExternalInput")
    y_out = nc.dram_tensor("y_out", [DO_SH, T], F32, kind="ExternalOutput")
    if debug:
        hd = [nc.dram_tensor(f"hd{k}", [128, L * DT * T], FP16, kind="ExternalOutput")
              for k in range(E_LOC)]
        ed = [nc.dram_tensor(f"ed{k}", [128, DT * T], FP16, kind="ExternalOutput")
              for k in range(E_LOC)]
        ffd = [nc.dram_tensor(f"ffd{k}", [128, NFT * T], FP16, kind="ExternalOutput")
               for k in range(E_LOC)]
        zd = nc.dram_tensor("zd", [128, 2 * T], FP16, kind="ExternalOutput")
        whd = nc.dram_tensor("whd", [128, L * 16 * 128], FP16, kind="ExternalOutput")
        w0d = nc.dram_tensor("w0d", [D_IN + 1, D], FP16, kind="ExternalOutput")
        xd = nc.dram_tensor("xd", [D_IN + 1, T], FP16, kind="ExternalOutput")

    rs_in = [nc.dram_tensor(f"rs_in{m}", [H_DEC, T], FP16) for m in range(N_DEC - 1)]
    rs_out = [nc.dram_tensor(f"rs_out{m}", [HD_SH, T], FP16)
              for m in range(N_DEC - 1)]
    rsy_in = nc.dram_tensor("rsy_in", [D_OUT, T], F32)
    rsy_out = nc.dram_tensor("rsy_out", [DO_SH, T], F32)
    RG = [list(range(N_CORES))]

    def whcol(l, i, j):
        return ((l * 16) + i * 4 + j) * 128

    def wincol(l, i, j):
        return ((l - 1) * 16 * 128) + (i * 4 + j) * 128

    with ExitStack() as ctx:
        ec = ctx.enter_context

        s_ld = Ctr(ec(nc.semaphore("s_ld")))
        s_mm = Ctr(ec(nc.semaphore("s_mm")))
        s_act = Ctr(ec(nc.semaphore("s_act")))
        s_st = Ctr(ec(nc.semaphore("s_st")))
        s_cc = Ctr(ec(nc.semaphore("s_cc")))
        s_dbg = Ctr(ec(nc.semaphore("s_dbg")))

        # ---- SBUF: left = long-lived ------------------------------------
        xT_sb = ec(nc.sbuf_tensor("xT_sb", [D_IN + 1, T], FP16))
        ones_sb = ec(nc.sbuf_tensor("ones_sb", [1, T], FP16))
        win0_sb = [ec(nc.sbuf_tensor(f"win0_sb{k}", [D_IN + 1, D], FP16))
                   for k in range(E_LOC)]
        brow_sb = [ec(nc.sbuf_tensor(f"brow_sb{k}", [1, (L - 1) * D], FP16))
                   for k in range(E_LOC)]
        bc_sb = ec(nc.sbuf_tensor("bc_sb", [128, NBC], F32))
        hb = [ec(nc.sbuf_tensor(f"hb{k}", [128, L * DT * T], FP16))
              for k in range(E_LOC)]
        ffs = [ec(nc.sbuf_tensor(f"ffs{k}", [128, NFT * T], FP16))
               for k in range(E_LOC)]
        enc_sb = [ec(nc.sbuf_tensor(f"enc{k}", [128, DT * T], FP16))
                  for k in range(E_LOC)]
        wff1_sb = [ec(nc.sbuf_tensor(f"wff1_sb{k}", [128, NCAT * NFT * 128], FP16))
                   for k in range(E_LOC)]
        wff2_sb = [ec(nc.sbuf_tensor(f"wff2_sb{k}", [128, NFT * DT * 128], FP16))
                   for k in range(E_LOC)]

        # ---- SBUF: right = 52KB arena, phase-aliased --------------------
        ARENA = 52 * 1024
        arena = ec(nc.sbuf_tensor("arena", [128, ARENA], U8, side="right"))
        abase = nc.lookup_mloc(arena).addr

        def at(name, shape, dtype, off):
            return nc.alloc_sbuf_tensor_at(name, shape, dtype, offset=abase + off)

        # phase 1 (recurrence): wh (12KB x2) + win (8KB x2) = 40KB
        wh_sb = [at(f"wh_sb{k}", [128, L * 16 * 128], FP16, k * 12 * 1024)
                 for k in range(E_LOC)]
        win_sb = [at(f"win_sb{k}", [128, (L - 1) * 16 * 128], FP16,
                     24 * 1024 + k * 8 * 1024) for k in range(E_LOC)]
        # phase 2 (decoder weights; DMA gated on recurrence completion)
        wd0_sb = at("wd0_sb", [128, NDK * 2 * 128], FP16, 0)            # 32KB
        wdm_sb = [at(f"wdm_sb{m}", [128, NHD * 2 * 128], FP16,
                     32 * 1024 + m * 8 * 1024) for m in range(N_DEC - 2)]  # 8KB x2
        wdo_sb = at("wdo_sb", [128, NHD * 128], FP16, 48 * 1024)        # 4KB

        # decoder activations alias the FF1[0] weight region (dead after FF1)
        wbase = nc.lookup_mloc(wff1_sb[0]).addr

        def at_w(name, shape, dtype, off):
            return nc.alloc_sbuf_tensor_at(name, shape, dtype, offset=wbase + off)

        zp_sb = at_w("zp_sb", [128, NHD * T], FP16, 0)                  # 4KB
        yp_sb = at_w("yp_sb", [128, (D_OUT // 128) * T], F32, 4 * 1024)  # 4KB
        zloc_sb = at_w("zloc_sb", [128, 2 * T], FP16, 8 * 1024)         # 512B
        ysb = at_w("ysb", [DO_SH, T], F32, 9 * 1024)                    # 512B

        # ---- PSUM -------------------------------------------------------
        pu = [ec(nc.psum_tensor(f"pu{k}", [128, DT * T], F32)) for k in range(E_LOC)]
        psb = [ec(nc.psum_tensor(f"psb{g}", [128, 512], F32)) for g in range(4)]

        def bff1_ap(k, j):
            return bc_sb[:, k * NFT + j:k * NFT + j + 1]

        def bff2_ap(k, j):
            c = 2 * NFT + k * DT + j
            return bc_sb[:, c:c + 1]

        def bz0_ap(j):
            c = 2 * NFT + 2 * DT + j
            return bc_sb[:, c:c + 1]

        def bzm_ap(m, j):
            c = 2 * NFT + 2 * DT + NHD + m * NHD + j
            return bc_sb[:, c:c + 1]

        def bzy_ap(j):
            c = 2 * NFT + 2 * DT + 3 * NHD + j
            return bc_sb[:, c:c + 1]

        thr = {}

        with nc.Block() as block:

            @block.sync
            def _(sync):
                def load(dst, src):
                    return s_ld.inc(sync.dma_start(dst, src), 16)

                load(xT_sb[:, :], xT_aug[:, :])
                load(ones_sb[:, :], ones_row[:, :])
                load(bc_sb[:, :], bcols[:, :])
                for k in range(E_LOC):
                    load(win0_sb[k][:, :], win0[k][:, :])
                    load(brow_sb[k][:, :], brow[k][:, :])
                for k in range(E_LOC):
                    thr[f"ld_whl0_{k}"] = load(wh_sb[k][:, 0:16 * 128],
                                               wh[k][:, 0:16 * 128])
                for k in range(E_LOC):
                    thr[f"ld_win{k}"] = load(win_sb[k][:, :], win[k][:, :])
                for l in (1, 2):
                    for k in range(E_LOC):
                        thr[f"ld_whl{l}_{k}"] = load(
                            wh_sb[k][:, l * 16 * 128:(l + 1) * 16 * 128],
                            wh[k][:, l * 16 * 128:(l + 1) * 16 * 128])
                for k in range(E_LOC):
                    thr[f"ld_wff1{k}"] = load(wff1_sb[k][:, :], wff1[k][:, :])
                for k in range(E_LOC):
                    thr[f"ld_wff2{k}"] = load(wff2_sb[k][:, :], wff2[k][:, :])
                # decoder weights alias wh/win: wait until the recurrence is done
                sync.wait_ge(s_act.h, E_LOC * L * T)
                thr["ld_wd0"] = load(wd0_sb[:, :], wd0[:, :])
                for m in range(N_DEC - 2):
                    load(wdm_sb[m][:, :], wdm[m][:, :])
                thr["ld_wdec"] = load(wdo_sb[:, :], wdo[:, :])

            @block.tensor
            def _(tensor):
                for l in range(L):
                    tensor.wait_ge(s_ld.h, thr[f"ld_whl{l}_1"])
                    for k in range(E_LOC):
                        if l == 0:
                            for j in range(DT):
                                mm = tensor.matmul(
                                    pu[k][:, j * T:(j + 1) * T],
                                    win0_sb[k][:, j * 128:(j + 1) * 128],
                                    xT_sb[:, :],
                                    start=(j == 0), stop=True,
                                    skip_group_check=True)
                        else:
                            if k == 0 and l == 1:
                                tensor.wait_ge(s_ld.h, thr["ld_win1"])
                            tensor.wait_ge(s_act.h, act_rec_thr(k, l - 1, T - 1))
                            for j in range(DT):
                                for i in range(DT):
                                    tensor.matmul(
                                        pu[k][:, j * T:(j + 1) * T],
                                        win_sb[k][:, wincol(l, i, j):wincol(l, i, j) + 128],
                                        hb[k][:, ((l - 1) * DT + i) * T:((l - 1) * DT + i + 1) * T],
                                        start=(j == 0 and i == 0), stop=False,
                                        skip_group_check=True)
                                mm = tensor.matmul(
                                    pu[k][:, j * T:(j + 1) * T],
                                    brow_sb[k][0:1, (l - 1) * D + j * 128:(l - 1) * D + (j + 1) * 128],
                                    ones_sb[0:1, :],
                                    start=False, stop=True,
                                    skip_group_check=True)
                        thr[f"mm_u_{k}_{l}"] = s_mm.inc(mm)

                    with nc.named_scope(f"rec{l}"):
                        for t in range(1, T):
                            for k in range(E_LOC):
                                tensor.wait_ge(s_act.h, act_rec_thr(k, l, t - 1))
                                for j in range(DT):
                                    for i in range(DT):
                                        mm = tensor.matmul(
                                            pu[k][:, j * T + t:j * T + t + 1],
                                            wh_sb[k][:, whcol(l, i, j):whcol(l, i, j) + 128],
                                            hb[k][:, (l * DT + i) * T + t - 1:(l * DT + i) * T + t],
                                            start=False, stop=(i == DT - 1),
                                            skip_group_check=True)
                                thr[f"mm_rec_{k}_{l}_{t}"] = s_mm.inc(mm)

                # --- FF1 -------------------------------------------------
                g = 0
                tensor.wait_ge(s_ld.h, thr["ld_wff11"])
                with nc.named_scope("ff1"):
                    for k in range(E_LOC):
                        tensor.wait_ge(s_act.h, act_rec_thr(k, L - 1, T - 1))
                        for j in range(NFT):
                            if g >= 4:
                                tensor.wait_ge(s_act.h, act_g_thr(g - 4))
                            for i in range(NCAT):
                                mm = tensor.matmul(
                                    psb[g % 4][:, 0:T],
                                    wff1_sb[k][:, (i * NFT + j) * 128:(i * NFT + j) * 128 + 128],
                                    hb[k][:, i * T:(i + 1) * T],
                                    start=(i == 0), stop=(i == NCAT - 1))
                            thr[f"mm_g{g}"] = s_mm.inc(mm)
                            g += 1

                # --- FF2 -------------------------------------------------
                tensor.wait_ge(s_ld.h, thr["ld_wff21"])
                with nc.named_scope("ff2"):
                    for k in range(E_LOC):
                        tensor.wait_ge(s_act.h, act_g_thr(k * NFT + NFT - 1))
                        for j in range(DT):
                            if g >= 4:
                                tensor.wait_ge(s_act.h, act_g_thr(g - 4))
                            for i in range(NFT):
                                mm = tensor.matmul(
                                    psb[g % 4][:, 0:T],
                                    wff2_sb[k][:, (i * DT + j) * 128:(i * DT + j) * 128 + 128],
                                    ffs[k][:, i * T:(i + 1) * T],
                                    start=(i == 0), stop=(i == NFT - 1))
                            thr[f"mm_g{g}"] = s_mm.inc(mm)
                            g += 1

                # --- decoder (k-sharded partials + ReduceScatter) --------
                tensor.wait_ge(s_ld.h, thr["ld_wd0"])
                with nc.named_scope("dec"):
                    # layer 0 partial: z0p[j] = sum_i enc_i^T tiles (8 local k)
                    tensor.wait_ge(s_act.h, act_g_thr(E_LOC * NFT + E_LOC * DT - 1))
                    for j in range(NHD):
                        if g >= 4:
                            tensor.wait_ge(s_act.h, act_g_thr(g - 4))
                        for i in range(2 * DT):
                            mm = tensor.matmul(
                                psb[g % 4][:, 0:T],
                                wd0_sb[:, (i * NHD + j) * 128:(i * NHD + j) * 128 + 128],
                                enc_sb[i // DT][:, (i % DT) * T:(i % DT + 1) * T],
                                start=(i == 0), stop=(i == 2 * DT - 1))
                        thr[f"mm_g{g}"] = s_mm.inc(mm)
                        g += 1

                    # mid layers: partial z_{m+1} from local 256-row block
                    for m in range(N_DEC - 2):
                        tensor.wait_ge(s_ld.h, thr["ld_wdec"])
                        tensor.wait_ge(s_act.h, th_tanh(m))
                        for j in range(NHD):
                            if g >= 4:
                                tensor.wait_ge(s_act.h, act_g_thr(g - 4))
                            for i in range(2):
                                mm = tensor.matmul(
                                    psb[g % 4][:, 0:T],
                                    wdm_sb[m][:, (i * NHD + j) * 128:(i * NHD + j) * 128 + 128],
                                    zloc_sb[:, i * T:(i + 1) * T],
                                    start=(i == 0), stop=(i == 1))
                            thr[f"mm_g{g}"] = s_mm.inc(mm)
                            g += 1

                    # out layer: partial y from local 256-row block
                    tensor.wait_ge(s_act.h, th_tanh(N_DEC - 2))
                    for j in range(D_OUT // 128):
                        if g >= 4:
                            tensor.wait_ge(s_act.h, act_g_thr(g - 4))
                        for i in range(2):
                            mm = tensor.matmul(
                                psb[g % 4][:, 0:T],
                                wdo_sb[:, (i * (D_OUT // 128) + j) * 128:(i * (D_OUT // 128) + j) * 128 + 128],
                                zloc_sb[:, i * T:(i + 1) * T],
                                start=(i == 0), stop=(i == 1))
                        thr[f"mm_g{g}"] = s_mm.inc(mm)
                        g += 1
                    thr["g_end"] = g

            @block.scalar
            def _(scalar):
                for l in range(L):
                    for t in range(T):
                        for k in range(E_LOC):
                            if t == 0:
                                scalar.wait_ge(s_mm.h, thr[f"mm_u_{k}_{l}"])
                            else:
                                scalar.wait_ge(s_mm.h, thr[f"mm_rec_{k}_{l}_{t}"])
                            a = scalar.activation(
                                hb[k][:, l * DT * T + t::T][:, 0:DT],
                                pu[k][:, t::T],
                                AF.Tanh)
                            s_act.inc(a)

                gg = 0
                for k in range(E_LOC):
                    for j in range(NFT):
                        scalar.wait_ge(s_mm.h, thr[f"mm_g{gg}"])
                        a = scalar.activation(
                            ffs[k][:, j * T:(j + 1) * T], psb[gg % 4][:, 0:T],
                            AF.Gelu_apprx_tanh, bias=bff1_ap(k, j))
                        s_act.inc(a)
                        gg += 1

                for k in range(E_LOC):
                    for j in range(DT):
                        scalar.wait_ge(s_mm.h, thr[f"mm_g{gg}"])
                        a = scalar.activation(
                            enc_sb[k][:, j * T:(j + 1) * T], psb[gg % 4][:, 0:T],
                            AF.Identity, bias=bff2_ap(k, j))
                        s_act.inc(a)
                        gg += 1

                # decoder shard rounds: partial ACTs -> store -> RS -> load -> tanh
                for j in range(NHD):
                    scalar.wait_ge(s_mm.h, thr[f"mm_g{gg}"])
                    a = scalar.activation(
                        zp_sb[:, j * T:(j + 1) * T], psb[gg % 4][:, 0:T],
                        AF.Identity, bias=bz0_ap(j))
                    s_act.inc(a)
                    gg += 1
                scalar.wait_ge(s_act.h, s_act.v)
                d = scalar.dma_start(
                    rs_in[0][:, :].rearrange("(j p) t -> p j t", p=128),
                    zp_sb[:, :])
                thr["st_z0"] = s_st.inc(d, 16)
                assert thr["st_z0"] == st_z(0)

                for m in range(N_DEC - 2):
                    scalar.wait_ge(s_cc.h, m + 1)
                    d = scalar.dma_start(
                        zp_sb[:, 0:2 * T],
                        rs_out[m][:, :].rearrange("(i p) t -> p i t", p=128))
                    s_st.inc(d, 16)
                    assert s_st.v == st_zf(m)
                    scalar.wait_ge(s_st.h, s_st.v)
                    a = scalar.activation(zloc_sb[:, :], zp_sb[:, 0:2 * T], AF.Tanh)
                    s_act.inc(a)
                    assert s_act.v == th_tanh(m)
                    for j in range(NHD):
                        scalar.wait_ge(s_mm.h, thr[f"mm_g{gg}"])
                        a = scalar.activation(
                            zp_sb[:, j * T:(j + 1) * T], psb[gg % 4][:, 0:T],
                            AF.Identity, bias=bzm_ap(m, j))
                        s_act.inc(a)
                        gg += 1
                    scalar.wait_ge(s_act.h, s_act.v)
                    d = scalar.dma_start(
                        rs_in[m + 1][:, :].rearrange("(j p) t -> p j t", p=128),
                        zp_sb[:, :])
                    thr[f"st_z{m + 1}"] = s_st.inc(d, 16)
                    assert thr[f"st_z{m + 1}"] == st_z(m + 1)

                scalar.wait_ge(s_cc.h, N_DEC - 1)
                d = scalar.dma_start(
                    zp_sb[:, 0:2 * T],
                    rs_out[N_DEC - 2][:, :].rearrange("(i p) t -> p i t", p=128))
                s_st.inc(d, 16)
                assert s_st.v == st_zf(N_DEC - 2)
                scalar.wait_ge(s_st.h, s_st.v)
                a = scalar.activation(zloc_sb[:, :], zp_sb[:, 0:2 * T], AF.Tanh)
                s_act.inc(a)
                assert s_act.v == th_tanh(N_DEC - 2)

                # y partials (fp32, per-core bias slot) -> store -> RS -> y_out
                for j in range(D_OUT // 128):
                    scalar.wait_ge(s_mm.h, thr[f"mm_g{gg}"])
                    a = scalar.activation(
                        yp_sb[:, j * T:(j + 1) * T], psb[gg % 4][:, 0:T],
                        AF.Identity, bias=bzy_ap(j))
                    s_act.inc(a)
                    gg += 1
                scalar.wait_ge(s_act.h, s_act.v)
                d = scalar.dma_start(
                    rsy_in[:, :].rearrange("(j p) t -> p j t", p=128),
                    yp_sb[:, :])
                s_st.inc(d, 16)
                assert s_st.v == ST_ZY
                scalar.wait_ge(s_cc.h, N_DEC)
                d = scalar.dma_start(y_out[:, :], rsy_out[:, :])
                s_st.inc(d, 16)
                assert s_st.v == ST_Y
                scalar.wait_ge(s_st.h, s_st.v)  # drain final store

            @block.gpsimd
            def _(gpsimd):
                if debug:
                    gpsimd.wait_ge(s_ld.h, thr["ld_whl2_1"])
                    s_dbg.inc(gpsimd.dma_start(whd[:, :], wh_sb[0][:, :]), 16)
                ADD = mybir.AluOpType.add
                for m in range(N_DEC - 1):
                    gpsimd.wait_ge(s_st.h, thr[f"st_z{m}"])
                    cc = gpsimd.collective_compute(
                        "ReduceScatter", ADD, replica_groups=RG,
                        ins=[rs_in[m][:, :]], outs=[rs_out[m][:, :]])
                    s_cc.inc(cc)
                gpsimd.wait_ge(s_st.h, ST_ZY)
                cc = gpsimd.collective_compute(
                    "ReduceScatter", ADD, replica_groups=RG,
                    ins=[rsy_in[:, :]], outs=[rsy_out[:, :]])
                s_cc.inc(cc)
                if debug:
                    gpsimd.wait_ge(s_st.h, ST_Y)
                    for k in range(E_LOC):
                        s_dbg.inc(gpsimd.dma_start(hd[k][:, :], hb[k][:, :]), 16)
                        s_dbg.inc(gpsimd.dma_start(ed[k][:, :], enc_sb[k][:, :]), 16)
                        s_dbg.inc(gpsimd.dma_start(ffd[k][:, :], ffs[k][:, :]), 16)
                    s_dbg.inc(gpsimd.dma_start(zd[:, :], zloc_sb[:, :]), 16)
                    s_dbg.inc(gpsimd.dma_start(w0d[:, :], win0_sb[0][:, :]), 16)
                    s_dbg.inc(gpsimd.dma_start(xd[:, :], xT_sb[:, :]), 16)
                    gpsimd.wait_ge(s_dbg.h, s_dbg.v)

        nc.compile()
    return nc


def prep_inputs(inputs):
    """Build the 8 per-core input maps from full numpy inputs (all fp16)."""
    f32 = lambda a: np.asarray(a, np.float32)
    F16 = np.float16
    x = f32(inputs["x"])
    W_in0, Wh0, b0 = f32(inputs["W_in0"]), f32(inputs["Wh0"]), f32(inputs["b0"])
    W_in_rest, Wh_rest, b_rest = (f32(inputs["W_in_rest"]), f32(inputs["Wh_rest"]),
                                  f32(inputs["b_rest"]))
    W_ff1, b_ff1 = f32(inputs["W_ff1"]), f32(inputs["b_ff1"])
    W_ff2, b_ff2 = f32(inputs["W_ff2"]), f32(inputs["b_ff2"])
    W_d0, b_d0 = f32(inputs["W_d0"]), f32(inputs["b_d0"])
    W_dmid, b_dmid = f32(inputs["W_dmid"]), f32(inputs["b_dmid"])
    W_dout, b_dout = f32(inputs["W_dout"]), f32(inputs["b_dout"])

    xT_aug = np.concatenate([x[0].T, np.ones((1, T), np.float32)], 0)  # [33, T]
    in_maps = []
    for c in range(N_CORES):
        m = {"xT_aug": xT_aug.astype(F16),
             "ones_row": np.ones((1, T), F16)}
        for k in range(E_LOC):
            e = E_LOC * c + k
            m[f"win0_{k}"] = np.concatenate(
                [W_in0[e], b0[e][None, :]], 0).astype(F16)       # [33, 512]
            wh_all = np.concatenate([Wh0[e][None], Wh_rest[e]], 0)  # [3, D, D]
            m[f"wh_{k}"] = np.concatenate(
                [_tile_kxm(wh_all[l]) for l in range(L)], axis=1).astype(F16)
            m[f"win_{k}"] = np.concatenate(
                [_tile_kxm(W_in_rest[e][l]) for l in range(L - 1)], axis=1).astype(F16)
            m[f"brow_{k}"] = b_rest[e].reshape(1, (L - 1) * D).astype(F16)
            m[f"wff1_{k}"] = _tile_kxm(W_ff1[e]).astype(F16)
            m[f"wff2_{k}"] = _tile_kxm(W_ff2[e]).astype(F16)
        bc = np.zeros((128, NBC), np.float32)
        for k in range(E_LOC):
            e = E_LOC * c + k
            bc[:, k * NFT:(k + 1) * NFT] = _bias_cols(b_ff1[e])
            bc[:, 2 * NFT + k * DT:2 * NFT + (k + 1) * DT] = _bias_cols(b_ff2[e])
        o = 2 * NFT + 2 * DT
        # decoder biases: core c contributes the bias for its own RS shard only
        for j in (2 * c, 2 * c + 1):
            bc[:, o + j] = b_d0[j * 128:(j + 1) * 128]
            for mm_ in range(N_DEC - 2):
                bc[:, o + NHD + mm_ * NHD + j] = b_dmid[mm_][j * 128:(j + 1) * 128]
        bc[:, o + 3 * NHD + c] = b_dout[c * 128:(c + 1) * 128]
        m["bcols"] = bc
        m["wd0"] = _tile_kxm(W_d0[c * E_LOC * D_ENC:(c + 1) * E_LOC * D_ENC, :]).astype(F16)
        for mm_ in range(N_DEC - 2):
            m[f"wdm{mm_}"] = _tile_kxm(
                W_dmid[mm_][c * HD_SH:(c + 1) * HD_SH, :]).astype(F16)
        m["wdo"] = _tile_kxm(W_dout[c * HD_SH:(c + 1) * HD_SH, :]).astype(F16)
        in_maps.append(m)
    return in_maps


def run(inputs, t_steps=T, trace=False, debug=False):
    from concourse.bass_utils import run_bass_kernel_spmd

    nc = build_nc(debug=debug)
    in_maps = prep_inputs(inputs)
    res = run_bass_kernel_spmd(nc, in_maps, list(range(N_CORES)), trace=trace)
    parts = [res.results[c]["y_out"] for c in range(N_CORES)]  # each [128, T]
    y = np.concatenate([np.asarray(p, np.float32).T for p in parts], axis=1)
    return y[None], res


def kernel(**inputs):
    y, _ = run(inputs, T, trace=False)
    return y
